# revision 8
# baseline (speedup 1.0000x reference)
"""DeepRC segment-softmax attention pooling kernel for 8 Trainium2 NeuronCores.

Strategy v2 (balanced shard, zero collectives, bf16 datapath):
  - N=131072 sorted instances split evenly: core c gets rows
    [c*16384, (c+1)*16384) -- no padding (vs per-bag pad to 18432 before).
    Bag boundaries fall inside at most one 512-subtile per boundary; those
    subtiles are split on host: kernel emits BOTH bagA-masked stats (zA,
    pooledA) and unmasked totals (z, pooled); host assigns B-side = tot - A.
  - xt is pre-transposed to [(l,c)=736 rows, 16384 cols] bf16; one core
    macrotile (2048 cols) loads with 2 batched DMAs (vs 6 unbatched f32).
  - Conv1d(K=32,C=23,KS=9,L=32->24) stays a banded matmul in bf16: 20
    (t,s) blocks of [128x128]; PSUM [128, 6, 256] per half-subtile.
  - maxpool over l: DVE reduces blocks 0-2, Pool max-chains blocks 3-5,
    DVE pair-maxes into bf16; partition fold 128->32 via Act copies + DVE
    bf16 maxes (2x DVE mode).
  - SELU split: Act relu + Act exp + min/add scalar_tensor_tensor on Pool;
    selu constant -lam*alpha deferred to host biases/output.
  - Attention logits use m=0 (|att| ~ O(1), exp safe in f32); host combine
    is exact in float64, so no per-subtile max is needed at all.
  - Attention MLP runs block-diagonal bf16 [128,512] per macrotile.
"""

import os
import sys

for _p in (
    "/root/.axon_site",
    "/root/.axon_site/_ro/trn_rl_repo",
    "/root/.axon_site/_ro/pypackages",
    "/opt/trn_rl_repo",
):
    if os.path.isdir(_p) and _p not in sys.path:
        sys.path.append(_p)

import numpy as np

import concourse.bass as bass
import concourse.mybir as mybir
from concourse.tile import TileContext, ScopedClock
from concourse.bass_utils import run_bass_kernel_spmd

AF = mybir.ActivationFunctionType
OP = mybir.AluOpType
AX = mybir.AxisListType
F32 = mybir.dt.float32
BF16 = mybir.dt.bfloat16
NP_BF16 = mybir.dt.np(mybir.dt.bfloat16)

# ---------------------------------------------------------------- constants
N_BAGS = 8
N_CORES = 8
L, C, K, U, KS = 32, 23, 32, 32, 9
LO = L - KS + 1            # 24 output positions
R = L * C                  # 736 rows of xT
NT = 6                     # PSUM M blocks (each 4 l x 32 k)
FD = 512                   # instances per subtile
HFD = 256                  # half-subtile (PSUM bank sizing)
QS = 4                     # subtiles stacked per macrotile
MACRO = QS * FD            # 2048

LAM = 1.0507009873554805
ALPHA = 1.6732632423543772
LA = LAM * ALPHA
LN_LA = float(np.log(LA))
C_SELU = -LA               # deferred selu constant

# ------------------------------------------------------- walrus workarounds


def _patched_drain_and_barrier(self, tick_clock, wait_clock):
    # stock version puts every outstanding sem wait on one drain; this
    # walrus build allows a single sync wait per instruction.
    nc = self.nc
    drain_inst = nc.sync.drain()
    wait_clock.add_sem_waits(
        drain_inst.ins, ScopedClock({None: tick_clock.global_clock})
    )
    si = drain_inst.ins.sync_info
    waits = list(si.on_wait or []) if si is not None else []
    if len(waits) > 1:
        si.on_wait = waits[:1]
        for w in waits[1:]:
            extra = nc.sync.drain()
            esi = extra.ins.sync_info
            if esi is None:
                extra.ins.sync_info = mybir.SyncInfo(on_wait=[w], on_update=[])
            else:
                esi.on_wait = [w]
    nc.all_engine_barrier()
    assert self.sems is not None
    popped = nc._tile_sem_poison_stack.pop()
    assert popped is self._sem_poison
    nc.clear_and_free_semaphores(list(self.sems.allocated().values()))
    nc.all_engine_barrier()


TileContext._drain_and_barrier = _patched_drain_and_barrier

_WSPLIT_CTR = [0]


def _split_multi_waits(nc):
    # move extra sem waits onto same-engine NoOps inserted just before the
    # owning instruction (equivalent gating, one wait per instruction).
    for func in nc.m.functions:
        for blk in func.blocks:
            out = []
            changed = False
            for inst in blk.instructions:
                si = inst.sync_info
                if si is not None and si.on_wait is not None and len(si.on_wait) > 1:
                    waits = list(si.on_wait)
                    for w in waits[:-1]:
                        _WSPLIT_CTR[0] += 1
                        nop = mybir.InstNoOp(
                            name=f"I-wsplit-{_WSPLIT_CTR[0]}", ins=[], outs=[]
                        )
                        nop.engine = inst.engine
                        nop.sync_info = mybir.SyncInfo(on_wait=[w], on_update=[])
                        out.append(nop)
                    si.on_wait = [waits[-1]]
                    changed = True
                out.append(inst)
            if changed:
                blk.instructions[:] = out
    return nc


# ------------------------------------------------------------- conv blocks


def _conv_block_list():
    """Nonzero (t, s) blocks of the banded weight matrix, t-major."""
    blocks = []
    for t in range(NT):
        lo_row = 23 * (4 * t)            # first needed row: l' = 4t
        hi_row = 23 * (4 * t + 12) + 22  # last needed row: l' = 4t+12, c=22
        s_lo, s_hi = lo_row // 128, hi_row // 128
        for s in range(s_lo, min(s_hi, 5) + 1):
            blocks.append((t, s))
    return blocks


CONV_BLOCKS = _conv_block_list()          # 20 blocks
N_CB = len(CONV_BLOCKS)


def _build_w2t(conv_w):
    w2t = np.zeros((768, 768), np.float32)
    for l in range(LO):
        for j in range(KS):
            lp = l + j
            # rows 23*lp .. +23 ; cols 32*l .. +32 ; value w[k, c, j]
            w2t[23 * lp : 23 * lp + 23, 32 * l : 32 * l + 32] = conv_w[:, :, j].T
    return w2t


# --------------------------------------------------------------- program


def _build_program(NPAD):
    T = NPAD // MACRO
    nc = bass.Bass()
    xt_d = nc.declare_dram_parameter("xt", [R, NPAD], BF16, isOutput=False)
    wconv_d = nc.declare_dram_parameter("wconv", [128, N_CB * 128], BF16, isOutput=False)
    wmat_d = nc.declare_dram_parameter("wmat", [128, 388], BF16, isOutput=False)
    wbias_d = nc.declare_dram_parameter("wbias", [128, 6], F32, isOutput=False)
    mask_d = nc.declare_dram_parameter("maskp", [QS, T * FD], BF16, isOutput=False)
    z_out = nc.declare_dram_parameter("z_out", [QS, T], F32, isOutput=True)
    za_out = nc.declare_dram_parameter("za_out", [QS, T], F32, isOutput=True)
    pooled_out = nc.declare_dram_parameter("pooled_out", [128, T], F32, isOutput=True)
    pooleda_out = nc.declare_dram_parameter("pooleda_out", [128, T], F32, isOutput=True)

    with TileContext(nc) as tc:
        with (
            tc.tile_pool(name="wpool", bufs=1) as wpool,
            tc.tile_pool(name="xpool", bufs=2) as xpool,
            tc.tile_pool(name="spool", bufs=3) as spool,
            tc.tile_pool(name="cpsum", bufs=2, space="PSUM") as cpsum,
            tc.tile_pool(name="mpsum", bufs=2, space="PSUM") as mpsum,
        ):
            wsb = wpool.tile([128, N_CB * 128], BF16)
            nc.sync.dma_start(wsb[:], wconv_d[:])
            wmat = wpool.tile([128, 388], BF16)
            nc.sync.dma_start(wmat[:], wmat_d[:])
            wbias = wpool.tile([128, 6], F32)
            nc.sync.dma_start(wbias[:], wbias_d[:])
            mask_sb = wpool.tile([QS, T * FD], BF16)
            nc.sync.dma_start(mask_sb[:], mask_d[:])
            z_sb = wpool.tile([QS, T], F32)
            za_sb = wpool.tile([QS, T], F32)
            pooled_sb = wpool.tile([128, T], F32)
            pooleda_sb = wpool.tile([128, T], F32)

            w1bd = wmat[:, 0:128]
            w2bd = wmat[:, 128:256]
            w3bd = wmat[:, 256:260]
            bc4 = wmat[0:4, 260:388]
            be_exp = wbias[:, 0:1]
            be_relu = wbias[:, 1:2]
            bh1_exp = wbias[:, 2:3]
            bh1_relu = wbias[:, 3:4]
            bh2_exp = wbias[:, 4:5]
            bh2_relu = wbias[:, 5:6]

            for j in range(T):
                xts = xpool.tile([128, NT, MACRO], BF16, tag="xts")
                col0 = j * MACRO
                # per-block 2D loads (3D dram->sbuf DMAs scramble on hw);
                # 2048-wide transfers keep descriptor count low
                for s in range(5):
                    nc.sync.dma_start(
                        xts[:, s, :],
                        xt_d[128 * s : 128 * (s + 1), col0 : col0 + MACRO],
                    )
                nc.sync.dma_start(
                    xts[0:96, 5, :], xt_d[640:736, col0 : col0 + MACRO]
                )
                er4 = spool.tile([128, FD], BF16, tag="er4")
                for q in range(QS):
                    bm = spool.tile([128, FD], BF16, tag="bm")
                    for cc in range(2):
                        c0 = q * FD + cc * HFD
                        ps = cpsum.tile([128, NT, HFD], F32, tag="cps")
                        for t in range(NT):
                            slist = [s for (tt, s) in CONV_BLOCKS if tt == t]
                            for ki, s in enumerate(slist):
                                idx = CONV_BLOCKS.index((t, s))
                                nc.tensor.matmul(
                                    ps[:, t, :],
                                    wsb[:, idx * 128 : (idx + 1) * 128],
                                    xts[:, s, c0 : c0 + HFD],
                                    start=(ki == 0),
                                    stop=(ki == len(slist) - 1),
                                )
                        # max over the 6 l-blocks: GPSIMD cannot read PSUM
                        # and only DVE/Act can, so Act exports blocks 4-5 to
                        # SBUF bf16 while DVE reduces 0-3; bf16 merges run
                        # at 2x on DVE.
                        c2 = spool.tile([128, 2, HFD], BF16, tag="c2")
                        nc.scalar.copy(c2[:], ps[:, 4:6, :])
                        bmr = spool.tile([128, HFD], BF16, tag="bmr")
                        nc.vector.tensor_reduce(
                            bmr[:], ps[:, 0:4, :].rearrange("p t f -> p f t"),
                            axis=AX.X, op=OP.max,
                        )
                        d2 = spool.tile([128, HFD], BF16, tag="d2")
                        nc.vector.tensor_max(d2[:], c2[:, 0, :], c2[:, 1, :])
                        nc.vector.tensor_max(
                            bm[:, cc * HFD : (cc + 1) * HFD], bmr[:], d2[:]
                        )
                    # partition fold 128 -> 32 (max over 4 l-residue groups)
                    t64 = spool.tile([64, FD], BF16, tag="t64")
                    nc.gpsimd.tensor_copy(t64[:], bm[64:128, :])
                    f1 = spool.tile([64, FD], BF16, tag="f1")
                    nc.vector.tensor_max(f1[:], bm[0:64, :], t64[:])
                    t32 = spool.tile([32, FD], BF16, tag="t32")
                    nc.gpsimd.tensor_copy(t32[:], f1[32:64, :])
                    nc.vector.tensor_max(
                        er4[32 * q : 32 * q + 32, :], f1[0:32, :], t32[:]
                    )

                # ---- selu(er4 + conv_b) (scaled branches, const deferred)
                t_relu = spool.tile([128, FD], F32, tag="t_relu")
                nc.scalar.activation(t_relu[:], er4[:], AF.Relu, bias=be_relu, scale=LAM)
                v_exp = spool.tile([128, FD], F32, tag="v_exp")
                nc.scalar.activation(v_exp[:], er4[:], AF.Exp, bias=be_exp, scale=1.0)
                e4 = spool.tile([128, FD], BF16, tag="e4")
                nc.vector.scalar_tensor_tensor(
                    e4[:], v_exp[:], LA, t_relu[:], op0=OP.min, op1=OP.add
                )
                # ---- MLP layer 1
                ps1 = mpsum.tile([128, FD], F32, tag="mlp")
                nc.tensor.matmul(ps1[:], w1bd, e4[:])
                t1 = spool.tile([128, FD], F32, tag="t1")
                nc.scalar.activation(t1[:], ps1[:], AF.Relu, bias=bh1_relu, scale=LAM)
                v1 = spool.tile([128, FD], F32, tag="v1")
                nc.scalar.activation(v1[:], ps1[:], AF.Exp, bias=bh1_exp, scale=1.0)
                h1 = spool.tile([128, FD], BF16, tag="h1")
                nc.vector.scalar_tensor_tensor(
                    h1[:], v1[:], LA, t1[:], op0=OP.min, op1=OP.add
                )
                # ---- MLP layer 2
                ps2 = mpsum.tile([128, FD], F32, tag="mlp")
                nc.tensor.matmul(ps2[:], w2bd, h1[:])
                t2 = spool.tile([128, FD], F32, tag="t2")
                nc.scalar.activation(t2[:], ps2[:], AF.Relu, bias=bh2_relu, scale=LAM)
                v2 = spool.tile([128, FD], F32, tag="v2")
                nc.scalar.activation(v2[:], ps2[:], AF.Exp, bias=bh2_exp, scale=1.0)
                h2 = spool.tile([128, FD], BF16, tag="h2")
                nc.vector.scalar_tensor_tensor(
                    h2[:], v2[:], LA, t2[:], op0=OP.min, op1=OP.add
                )
                # ---- attention logits (b3 cancels in softmax; m == 0)
                psa = mpsum.tile([4, FD], F32, tag="mlp")
                nc.tensor.matmul(psa[:], w3bd, h2[:])
                pexp = spool.tile([4, FD], BF16, tag="pexp")
                nc.scalar.activation(
                    pexp[:], psa[:], AF.Exp, bias=0.0, scale=1.0,
                    accum_out=z_sb[:, j : j + 1],
                )
                p4a = spool.tile([4, FD], BF16, tag="p4a")
                nc.vector.scalar_tensor_tensor(
                    p4a[:], pexp[:], 1.0, mask_sb[:, j * FD : (j + 1) * FD],
                    op0=OP.mult, op1=OP.mult, accum_out=za_sb[:, j : j + 1],
                )
                # ---- pooled(+A) += e4 * broadcast(p) per subtile group
                psbT = mpsum.tile([128, FD], F32, tag="mlp")
                nc.tensor.matmul(psbT[:], bc4, pexp[:])
                psbA = mpsum.tile([128, FD], F32, tag="mlp")
                nc.tensor.matmul(psbA[:], bc4, p4a[:])
                weT = spool.tile([128, FD], F32, tag="weT")
                nc.vector.scalar_tensor_tensor(
                    weT[:], e4[:], 1.0, psbT[:],
                    op0=OP.mult, op1=OP.mult, accum_out=pooled_sb[:, j : j + 1],
                )
                weA = spool.tile([128, FD], F32, tag="weA")
                nc.vector.scalar_tensor_tensor(
                    weA[:], e4[:], 1.0, psbA[:],
                    op0=OP.mult, op1=OP.mult, accum_out=pooleda_sb[:, j : j + 1],
                )

            nc.sync.dma_start(z_out[:], z_sb[:])
            nc.sync.dma_start(za_out[:], za_sb[:])
            nc.sync.dma_start(pooled_out[:], pooled_sb[:])
            nc.sync.dma_start(pooleda_out[:], pooleda_sb[:])

    _split_multi_waits(nc)
    return nc


_PROGRAM_CACHE = {}
LAST_RESULTS = None  # set by kernel(); test.py reads trace/exec info


def _get_program(NPAD):
    if NPAD not in _PROGRAM_CACHE:
        _PROGRAM_CACHE[NPAD] = _build_program(NPAD)
    return _PROGRAM_CACHE[NPAD]


# ----------------------------------------------------------------- kernel


def kernel(
    inputs,
    segment_ids,
    conv_w,
    conv_b,
    att_w1,
    att_b1,
    att_w2,
    att_b2,
    att_w3,
    att_b3,
    out_w,
    out_b,
):
    global LAST_RESULTS
    x = np.asarray(inputs, np.float32)
    seg = np.asarray(segment_ids)
    conv_w = np.asarray(conv_w, np.float32)
    conv_b = np.asarray(conv_b, np.float32)
    att_w1 = np.asarray(att_w1, np.float32)
    att_b1 = np.asarray(att_b1, np.float32)
    att_w2 = np.asarray(att_w2, np.float32)
    att_b2 = np.asarray(att_b2, np.float32)
    att_w3 = np.asarray(att_w3, np.float32)
    att_b3 = np.asarray(att_b3, np.float32)
    out_w = np.asarray(out_w, np.float32)
    out_b = np.asarray(out_b, np.float32)

    n_total = x.shape[0]
    NPAD = -(-n_total // (N_CORES * MACRO)) * MACRO   # per-core cols
    T = NPAD // MACRO
    n_padded = N_CORES * NPAD
    n_pad = n_padded - n_total

    # ---------------- weights (shared by all cores)
    w2t = _build_w2t(conv_w)
    wconv = np.zeros((128, N_CB * 128), np.float32)
    for idx, (t, s) in enumerate(CONV_BLOCKS):
        wconv[:, idx * 128 : (idx + 1) * 128] = w2t[
            128 * s : 128 * (s + 1), 128 * t : 128 * (t + 1)
        ]

    b1p = att_b1 + C_SELU * (att_w1 @ np.ones(K, np.float32))
    b2p = att_b2 + C_SELU * (att_w2 @ np.ones(U, np.float32))

    wmat = np.zeros((128, 388), np.float32)
    wbias = np.zeros((128, 6), np.float32)
    for q in range(QS):
        sl = slice(32 * q, 32 * q + 32)
        wmat[sl, 0:128][:, sl] = att_w1.T          # w1bd
        wmat[sl, 128:256][:, sl] = att_w2.T        # w2bd
        wmat[sl, 256 + q] = att_w3[0]              # w3bd
        wmat[q, 260 + 32 * q : 260 + 32 * q + 32] = 1.0  # bc4
        wbias[sl, 0] = conv_b + LN_LA
        wbias[sl, 1] = LAM * conv_b
        wbias[sl, 2] = b1p + LN_LA
        wbias[sl, 3] = LAM * b1p
        wbias[sl, 4] = b2p + LN_LA
        wbias[sl, 5] = LAM * b2p
    wconv16 = wconv.astype(NP_BF16)
    wmat16 = wmat.astype(NP_BF16)

    # ---------------- per-core inputs + bag bookkeeping
    xf = x.reshape(n_total, R)
    seg_pad = np.concatenate([seg, np.full(n_pad, N_BAGS, seg.dtype)])
    in_maps = []
    bagA = np.zeros((N_CORES, T, QS), np.int64)
    bagB = np.full((N_CORES, T, QS), -1, np.int64)
    npad_sub = np.zeros((N_CORES, T, QS), np.int64)
    for c in range(N_CORES):
        s0 = c * NPAD
        xt = np.zeros((R, NPAD), NP_BF16)
        real = min(NPAD, max(0, n_total - s0))
        if real > 0:
            xt[:, :real] = xf[s0 : s0 + real].T.astype(NP_BF16)
        ids = seg_pad[s0 : s0 + NPAD].reshape(T, QS, FD)
        first = ids[:, :, 0]
        # real-instance last bag per subtile (pad slots hold N_BAGS)
        real_mask = ids < N_BAGS
        last_real = np.where(real_mask, ids, -1).max(axis=2)
        if ((last_real - first) > 1).any() and (last_real >= 0).all():
            raise ValueError("subtile spans >2 bags; unsupported input shape")
        bagA[c] = np.where(first < N_BAGS, first, -1)
        hasB = (last_real > first) & (first < N_BAGS)
        bagB[c] = np.where(hasB, last_real, -1)
        npad_sub[c] = (~real_mask).sum(axis=2)
        maskA = (ids == first[:, :, None]).astype(np.float32)
        maskA *= real_mask  # pad slots excluded even if first is pad
        # layout [QS, T*FD]
        maskp = np.ascontiguousarray(
            maskA.transpose(1, 0, 2).reshape(QS, T * FD)
        ).astype(NP_BF16)
        in_maps.append(
            {
                "xt": xt,
                "wconv": wconv16,
                "wmat": wmat16,
                "wbias": wbias,
                "maskp": maskp,
            }
        )

    nc = _get_program(NPAD)
    trace_mode = int(os.environ.get("DEEPRC_TRACE", "0"))
    kwargs = {}
    if trace_mode == 1:
        kwargs = dict(trace=True, trace_cores=[0])
    elif trace_mode >= 2:
        kwargs = dict(trace=True, trace_cores=list(range(N_CORES)), stitch_traces=True)
    res = run_bass_kernel_spmd(
        nc,
        in_maps,
        core_ids=list(range(N_CORES)),
        **kwargs,
    )
    LAST_RESULTS = res

    # ---------------- pad-instance constants (host, float64-exact path)
    # a zero input row gives er4 = 0 (conv of zeros, max of zeros), so the
    # device computes for each pad slot: e_pad = selu(conv_b)+LA etc.  The
    # unmasked totals z/pooled include those; subtract exactly here.
    if n_pad > 0:
        e_pad = np.where(
            conv_b > 0, LAM * conv_b, LA * np.exp(conv_b) - LA
        ) + LA  # selu(conv_b) + LA, shape [K]
        hh = np.where(
            att_w1 @ (e_pad + C_SELU) + att_b1 > 0,
            LAM * (att_w1 @ (e_pad + C_SELU) + att_b1),
            LA * np.exp(att_w1 @ (e_pad + C_SELU) + att_b1) - LA,
        )
        hh2 = np.where(
            att_w2 @ hh + att_b2 > 0,
            LAM * (att_w2 @ hh + att_b2),
            LA * np.exp(att_w2 @ hh + att_b2) - LA,
        )
        att_pad = float(att_w3[0] @ hh2)
        pz_pad = float(np.exp(att_pad))
        ppool_pad = pz_pad * e_pad  # [K]
    else:
        pz_pad = 0.0
        ppool_pad = np.zeros(K)

    # ---------------- exact host combine (float64)
    Z = np.zeros(N_BAGS, np.float64)
    P = np.zeros((N_BAGS, K), np.float64)
    for c in range(N_CORES):
        r = res.results[c]
        z = r["z_out"].astype(np.float64)            # [4, T]
        za = r["za_out"].astype(np.float64)          # [4, T]
        pooled = r["pooled_out"].astype(np.float64).reshape(QS, K, T)
        pooleda = r["pooleda_out"].astype(np.float64).reshape(QS, K, T)
        for j in range(T):
            for q in range(QS):
                bA = bagA[c, j, q]
                if bA < 0:
                    continue
                bB = bagB[c, j, q]
                if bB < 0:
                    Z[bA] += za[q, j]
                    P[bA] += pooleda[q, :, j]
                else:
                    Z[bA] += za[q, j]
                    P[bA] += pooleda[q, :, j]
                    npd = npad_sub[c, j, q]
                    Z[bB] += z[q, j] - za[q, j] - npd * pz_pad
                    P[bB] += (
                        pooled[q, :, j] - pooleda[q, :, j] - npd * ppool_pad
                    )

    out = np.zeros((N_BAGS, 1), np.float32)
    for b in range(N_BAGS):
        pooled_bag = P[b] / Z[b] + C_SELU
        out[b, 0] = np.float32(
            float(out_w.astype(np.float64)[0] @ pooled_bag) + float(out_b[0])
        )
    return out


# revision 20
# speedup vs baseline: 1.0757x; 1.0757x over previous
"""DeepRC segment-softmax attention pooling kernel for 8 Trainium2 NeuronCores.

Strategy v2 (balanced shard, zero collectives, bf16 datapath):
  - N=131072 sorted instances split evenly: core c gets rows
    [c*16384, (c+1)*16384) -- no padding (vs per-bag pad to 18432 before).
    Bag boundaries fall inside at most one 512-subtile per boundary; those
    subtiles are split on host: kernel emits BOTH bagA-masked stats (zA,
    pooledA) and unmasked totals (z, pooled); host assigns B-side = tot - A.
  - xt is pre-transposed to [(l,c)=736 rows, 16384 cols] bf16; one core
    macrotile (2048 cols) loads with 2 batched DMAs (vs 6 unbatched f32).
  - Conv1d(K=32,C=23,KS=9,L=32->24) stays a banded matmul in bf16: 20
    (t,s) blocks of [128x128]; PSUM [128, 6, 256] per half-subtile.
  - maxpool over l: DVE reduces blocks 0-2, Pool max-chains blocks 3-5,
    DVE pair-maxes into bf16; partition fold 128->32 via Act copies + DVE
    bf16 maxes (2x DVE mode).
  - SELU split: Act relu + Act exp + min/add scalar_tensor_tensor on Pool;
    selu constant -lam*alpha deferred to host biases/output.
  - Attention logits use m=0 (|att| ~ O(1), exp safe in f32); host combine
    is exact in float64, so no per-subtile max is needed at all.
  - Attention MLP runs block-diagonal bf16 [128,512] per macrotile.
"""

import os
import sys

for _p in (
    "/root/.axon_site",
    "/root/.axon_site/_ro/trn_rl_repo",
    "/root/.axon_site/_ro/pypackages",
    "/opt/trn_rl_repo",
):
    if os.path.isdir(_p) and _p not in sys.path:
        sys.path.append(_p)

import numpy as np

import concourse.bass as bass
import concourse.mybir as mybir
from concourse.tile import TileContext, ScopedClock
from concourse.bass_utils import run_bass_kernel_spmd

AF = mybir.ActivationFunctionType
OP = mybir.AluOpType
AX = mybir.AxisListType
F32 = mybir.dt.float32
BF16 = mybir.dt.bfloat16
F8E4 = mybir.dt.float8e4
PM = mybir.MatmulPerfMode
NP_BF16 = mybir.dt.np(mybir.dt.bfloat16)
NP_F8 = mybir.dt.np(mybir.dt.float8e4)

# ---------------------------------------------------------------- constants
N_BAGS = 8
N_CORES = 8
L, C, K, U, KS = 32, 23, 32, 32, 9
LO = L - KS + 1            # 24 output positions
R = L * C                  # 736 rows of xT
NT = 6                     # PSUM M blocks (each 4 l x 32 k)
FD = 512                   # instances per subtile
HFD = 256                  # half-subtile (PSUM bank sizing)
QS = 4                     # subtiles stacked per macrotile
MACRO = QS * FD            # 2048

LAM = 1.0507009873554805
ALPHA = 1.6732632423543772
LA = LAM * ALPHA
LN_LA = float(np.log(LA))
C_SELU = -LA               # deferred selu constant

# ------------------------------------------------------- walrus workarounds


def _patched_drain_and_barrier(self, tick_clock, wait_clock):
    # stock version puts every outstanding sem wait on one drain; this
    # walrus build allows a single sync wait per instruction.
    nc = self.nc
    drain_inst = nc.sync.drain()
    wait_clock.add_sem_waits(
        drain_inst.ins, ScopedClock({None: tick_clock.global_clock})
    )
    si = drain_inst.ins.sync_info
    waits = list(si.on_wait or []) if si is not None else []
    if len(waits) > 1:
        si.on_wait = waits[:1]
        for w in waits[1:]:
            extra = nc.sync.drain()
            esi = extra.ins.sync_info
            if esi is None:
                extra.ins.sync_info = mybir.SyncInfo(on_wait=[w], on_update=[])
            else:
                esi.on_wait = [w]
    nc.all_engine_barrier()
    assert self.sems is not None
    popped = nc._tile_sem_poison_stack.pop()
    assert popped is self._sem_poison
    nc.clear_and_free_semaphores(list(self.sems.allocated().values()))
    nc.all_engine_barrier()


TileContext._drain_and_barrier = _patched_drain_and_barrier

_WSPLIT_CTR = [0]


def _split_multi_waits(nc):
    # move extra sem waits onto same-engine NoOps inserted just before the
    # owning instruction (equivalent gating, one wait per instruction).
    for func in nc.m.functions:
        for blk in func.blocks:
            out = []
            changed = False
            for inst in blk.instructions:
                si = inst.sync_info
                if si is not None and si.on_wait is not None and len(si.on_wait) > 1:
                    waits = list(si.on_wait)
                    for w in waits[:-1]:
                        _WSPLIT_CTR[0] += 1
                        nop = mybir.InstNoOp(
                            name=f"I-wsplit-{_WSPLIT_CTR[0]}", ins=[], outs=[]
                        )
                        nop.engine = inst.engine
                        nop.sync_info = mybir.SyncInfo(on_wait=[w], on_update=[])
                        out.append(nop)
                    si.on_wait = [waits[-1]]
                    changed = True
                out.append(inst)
            if changed:
                blk.instructions[:] = out
    return nc


# ------------------------------------------------------------- conv blocks


def _conv_block_list():
    """Nonzero (t, s) blocks of the banded weight matrix, t-major."""
    blocks = []
    for t in range(NT):
        lo_row = 23 * (4 * t)            # first needed row: l' = 4t
        hi_row = 23 * (4 * t + 12) + 22  # last needed row: l' = 4t+12, c=22
        s_lo, s_hi = lo_row // 128, hi_row // 128
        for s in range(s_lo, min(s_hi, 5) + 1):
            blocks.append((t, s))
    return blocks


CONV_BLOCKS = _conv_block_list()          # 20 blocks
N_CB = len(CONV_BLOCKS)

# fp8 DoubleRow conv: stacked operand rows (x8[r], r8[r], x8[r]) for each
# original row r (3*736 = 2208 rows, padded to 18 chunks of 128 = 2304).
# Out-block t needs stacked rows [276t, 276t+828) -> 4 chunk-pairs each.
SROWS = 3 * R                  # 2208
SCHUNKS = 18                   # ceil(2208/128) -> 2304 rows
MSTART = [(276 * t) // 256 for t in range(NT)]   # first chunk-pair per t
N_DR = 4                       # chunk-pairs per out-block


def _build_w2t(conv_w):
    w2t = np.zeros((768, 768), np.float32)
    for l in range(LO):
        for j in range(KS):
            lp = l + j
            # rows 23*lp .. +23 ; cols 32*l .. +32 ; value w[k, c, j]
            w2t[23 * lp : 23 * lp + 23, 32 * l : 32 * l + 32] = conv_w[:, :, j].T
    return w2t


# --------------------------------------------------------------- program


def _build_program(NPAD):
    T = NPAD // MACRO
    nc = bass.Bass()
    sx_d = nc.declare_dram_parameter("sx", [128, SCHUNKS * NPAD], F8E4, isOutput=False)
    wconv_d = nc.declare_dram_parameter("wconv", [128, NT * N_DR * 256], F8E4, isOutput=False)
    wmat_d = nc.declare_dram_parameter("wmat", [128, 388], BF16, isOutput=False)
    wbias_d = nc.declare_dram_parameter("wbias", [128, 6], F32, isOutput=False)
    mask_d = nc.declare_dram_parameter("maskp", [QS, T * FD], BF16, isOutput=False)
    z_out = nc.declare_dram_parameter("z_out", [QS, T], F32, isOutput=True)
    za_out = nc.declare_dram_parameter("za_out", [QS, T], F32, isOutput=True)
    pooled_out = nc.declare_dram_parameter("pooled_out", [128, T], F32, isOutput=True)
    pooleda_out = nc.declare_dram_parameter("pooleda_out", [128, T], F32, isOutput=True)

    with TileContext(nc) as tc:
        with (
            tc.tile_pool(name="wpool", bufs=1) as wpool,
            tc.tile_pool(name="xpool", bufs=2) as xpool,
            tc.tile_pool(name="spool", bufs=3) as spool,
            tc.tile_pool(name="cpsum", bufs=2, space="PSUM") as cpsum,
            tc.tile_pool(name="mpsum", bufs=2, space="PSUM") as mpsum,
        ):
            wsb = wpool.tile([128, NT * N_DR * 256], F8E4)
            nc.sync.dma_start(wsb[:], wconv_d[:])
            wmat = wpool.tile([128, 388], BF16)
            nc.sync.dma_start(wmat[:], wmat_d[:])
            wbias = wpool.tile([128, 6], F32)
            nc.sync.dma_start(wbias[:], wbias_d[:])
            mask_sb = wpool.tile([QS, T * FD], BF16)
            nc.sync.dma_start(mask_sb[:], mask_d[:])
            z_sb = wpool.tile([QS, T], F32)
            za_sb = wpool.tile([QS, T], F32)
            pooled_sb = wpool.tile([128, T], F32)
            pooleda_sb = wpool.tile([128, T], F32)

            w1bd = wmat[:, 0:128]
            w2bd = wmat[:, 128:256]
            w3bd = wmat[:, 256:260]
            bc4 = wmat[0:4, 260:388]
            be_exp = wbias[:, 0:1]
            be_relu = wbias[:, 1:2]
            bh1_exp = wbias[:, 2:3]
            bh1_relu = wbias[:, 3:4]
            bh2_exp = wbias[:, 4:5]
            bh2_relu = wbias[:, 5:6]

            for j in range(T):
                xts = xpool.tile([128, SCHUNKS, MACRO], F8E4, tag="xts")
                col0 = j * MACRO
                # one partition-outer strided DMA per macrotile: chunk s of
                # the stacked fp8 operand lives at dram cols s*NPAD + col
                nc.sync.dma_start(
                    xts[:],
                    sx_d[:].rearrange("p (s f) -> p s f", s=SCHUNKS)[
                        :, :, col0 : col0 + MACRO
                    ],
                )
                er4 = spool.tile([128, FD], BF16, tag="er4")
                for q in range(QS):
                    bm = spool.tile([128, FD], BF16, tag="bm")
                    for cc in range(2):
                        c0 = q * FD + cc * HFD
                        ps = cpsum.tile([128, NT, HFD], F32, tag="cps")
                        for t in range(NT):
                            for mi in range(N_DR):
                                m = MSTART[t] + mi
                                idx = t * N_DR + mi
                                nc.tensor.matmul(
                                    ps[:, t, :],
                                    wsb[
                                        :, idx * 256 : (idx + 1) * 256
                                    ].rearrange("p (i m) -> p i m", i=2),
                                    xts[:, 2 * m : 2 * m + 2, c0 : c0 + HFD],
                                    start=(mi == 0),
                                    stop=(mi == N_DR - 1),
                                    perf_mode=PM.DoubleRow,
                                )
                        # max over the 6 l-blocks: GPSIMD cannot read PSUM,
                        # so Act exports blocks to SBUF bf16 and DVE merges
                        # at 2x; halves alternate the DVE/Act split to
                        # balance both engines.
                        bmo = bm[:, cc * HFD : (cc + 1) * HFD]
                        if cc == 0:
                            # DVE reduces blocks 0-1, Act copies 2-5
                            c4 = spool.tile([128, 4, HFD], BF16, tag="c4")
                            nc.scalar.copy(c4[:], ps[:, 2:6, :])
                            bmr = spool.tile([128, HFD], BF16, tag="bmr")
                            nc.vector.tensor_reduce(
                                bmr[:],
                                ps[:, 0:2, :].rearrange("p t f -> p f t"),
                                axis=AX.X, op=OP.max,
                            )
                            d2 = spool.tile([128, 2, HFD], BF16, tag="d2")
                            nc.vector.tensor_tensor(
                                d2[:], c4[:, 0:2, :], c4[:, 2:4, :], op=OP.max
                            )
                            d1 = spool.tile([128, HFD], BF16, tag="d1")
                            nc.vector.tensor_max(d1[:], d2[:, 0, :], d2[:, 1, :])
                            nc.vector.tensor_max(bmo, bmr[:], d1[:])
                        else:
                            # Act copies all 6 blocks, DVE merges the tree
                            c6 = spool.tile([128, 6, HFD], BF16, tag="c6")
                            nc.scalar.copy(c6[:], ps[:])
                            e3 = spool.tile([128, 3, HFD], BF16, tag="e3")
                            nc.vector.tensor_tensor(
                                e3[:], c6[:, 0:3, :], c6[:, 3:6, :], op=OP.max
                            )
                            e1 = spool.tile([128, HFD], BF16, tag="e1")
                            nc.vector.tensor_max(e1[:], e3[:, 0, :], e3[:, 1, :])
                            nc.vector.tensor_max(bmo, e1[:], e3[:, 2, :])
                    # partition fold 128 -> 32 (max over 4 l-residue groups)
                    t64 = spool.tile([64, FD], BF16, tag="t64")
                    nc.gpsimd.tensor_copy(t64[:], bm[64:128, :])
                    f1 = spool.tile([64, FD], BF16, tag="f1")
                    nc.vector.tensor_max(f1[:], bm[0:64, :], t64[:])
                    t32 = spool.tile([32, FD], BF16, tag="t32")
                    nc.gpsimd.tensor_copy(t32[:], f1[32:64, :])
                    nc.vector.tensor_max(
                        er4[32 * q : 32 * q + 32, :], f1[0:32, :], t32[:]
                    )

                # ---- selu(er4/16 + conv_b): the fp8 conv runs at 16x scale,
                # folded into the activation scale (maxpool commutes)
                t_relu = spool.tile([128, FD], F32, tag="t_relu")
                nc.scalar.activation(
                    t_relu[:], er4[:], AF.Relu, bias=be_relu, scale=LAM / 16.0
                )
                v_exp = spool.tile([128, FD], F32, tag="v_exp")
                nc.scalar.activation(
                    v_exp[:], er4[:], AF.Exp, bias=be_exp, scale=1.0 / 16.0
                )
                e4 = spool.tile([128, FD], BF16, tag="e4")
                nc.vector.scalar_tensor_tensor(
                    e4[:], v_exp[:], LA, t_relu[:], op0=OP.min, op1=OP.add
                )
                # ---- MLP layer 1
                ps1 = mpsum.tile([128, FD], F32, tag="mlp")
                nc.tensor.matmul(ps1[:], w1bd, e4[:])
                t1 = spool.tile([128, FD], F32, tag="t1")
                nc.scalar.activation(t1[:], ps1[:], AF.Relu, bias=bh1_relu, scale=LAM)
                v1 = spool.tile([128, FD], F32, tag="v1")
                nc.scalar.activation(v1[:], ps1[:], AF.Exp, bias=bh1_exp, scale=1.0)
                h1 = spool.tile([128, FD], BF16, tag="h1")
                nc.vector.scalar_tensor_tensor(
                    h1[:], v1[:], LA, t1[:], op0=OP.min, op1=OP.add
                )
                # ---- MLP layer 2
                ps2 = mpsum.tile([128, FD], F32, tag="mlp")
                nc.tensor.matmul(ps2[:], w2bd, h1[:])
                t2 = spool.tile([128, FD], F32, tag="t2")
                nc.scalar.activation(t2[:], ps2[:], AF.Relu, bias=bh2_relu, scale=LAM)
                v2 = spool.tile([128, FD], F32, tag="v2")
                nc.scalar.activation(v2[:], ps2[:], AF.Exp, bias=bh2_exp, scale=1.0)
                h2 = spool.tile([128, FD], BF16, tag="h2")
                nc.vector.scalar_tensor_tensor(
                    h2[:], v2[:], LA, t2[:], op0=OP.min, op1=OP.add
                )
                # ---- attention logits (b3 cancels in softmax; m == 0)
                psa = mpsum.tile([4, FD], F32, tag="mlp")
                nc.tensor.matmul(psa[:], w3bd, h2[:])
                pexp = spool.tile([4, FD], BF16, tag="pexp")
                nc.scalar.activation(
                    pexp[:], psa[:], AF.Exp, bias=0.0, scale=1.0,
                    accum_out=z_sb[:, j : j + 1],
                )
                p4a = spool.tile([4, FD], BF16, tag="p4a")
                nc.vector.scalar_tensor_tensor(
                    p4a[:], pexp[:], 1.0, mask_sb[:, j * FD : (j + 1) * FD],
                    op0=OP.mult, op1=OP.mult, accum_out=za_sb[:, j : j + 1],
                )
                # ---- pooled(+A) += e4 * broadcast(p) per subtile group
                psbT = mpsum.tile([128, FD], F32, tag="mlp")
                nc.tensor.matmul(psbT[:], bc4, pexp[:])
                psbA = mpsum.tile([128, FD], F32, tag="mlp")
                nc.tensor.matmul(psbA[:], bc4, p4a[:])
                weT = spool.tile([128, FD], F32, tag="weT")
                nc.vector.scalar_tensor_tensor(
                    weT[:], e4[:], 1.0, psbT[:],
                    op0=OP.mult, op1=OP.mult, accum_out=pooled_sb[:, j : j + 1],
                )
                weA = spool.tile([128, FD], F32, tag="weA")
                nc.vector.scalar_tensor_tensor(
                    weA[:], e4[:], 1.0, psbA[:],
                    op0=OP.mult, op1=OP.mult, accum_out=pooleda_sb[:, j : j + 1],
                )

            nc.sync.dma_start(z_out[:], z_sb[:])
            nc.sync.dma_start(za_out[:], za_sb[:])
            nc.sync.dma_start(pooled_out[:], pooled_sb[:])
            nc.sync.dma_start(pooleda_out[:], pooleda_sb[:])

    _split_multi_waits(nc)
    return nc


_PROGRAM_CACHE = {}
LAST_RESULTS = None  # set by kernel(); test.py reads trace/exec info


def _get_program(NPAD):
    if NPAD not in _PROGRAM_CACHE:
        _PROGRAM_CACHE[NPAD] = _build_program(NPAD)
    return _PROGRAM_CACHE[NPAD]


# ----------------------------------------------------------------- kernel


def kernel(
    inputs,
    segment_ids,
    conv_w,
    conv_b,
    att_w1,
    att_b1,
    att_w2,
    att_b2,
    att_w3,
    att_b3,
    out_w,
    out_b,
):
    global LAST_RESULTS
    x = np.asarray(inputs, np.float32)
    seg = np.asarray(segment_ids)
    conv_w = np.asarray(conv_w, np.float32)
    conv_b = np.asarray(conv_b, np.float32)
    att_w1 = np.asarray(att_w1, np.float32)
    att_b1 = np.asarray(att_b1, np.float32)
    att_w2 = np.asarray(att_w2, np.float32)
    att_b2 = np.asarray(att_b2, np.float32)
    att_w3 = np.asarray(att_w3, np.float32)
    att_b3 = np.asarray(att_b3, np.float32)
    out_w = np.asarray(out_w, np.float32)
    out_b = np.asarray(out_b, np.float32)

    n_total = x.shape[0]
    NPAD = -(-n_total // (N_CORES * MACRO)) * MACRO   # per-core cols
    T = NPAD // MACRO
    n_padded = N_CORES * NPAD
    n_pad = n_padded - n_total

    # ---------------- weights (shared by all cores)
    # fp8 DoubleRow conv at 16x scale:
    #   16*w2t ~= W16 + Wr16 with W16 = fp8(16 w2t), Wr16 = fp8(W16 - 16 w2t)
    #   16*y   ~= W16 @ x8 + W16 @ r8 - Wr16 @ x8
    # stacked stationary rows (3r, 3r+1, 3r+2) = (W16[r], W16[r], -Wr16[r])
    w2t = _build_w2t(conv_w)
    w16 = 16.0 * w2t[:R]                               # [736, 768]
    W16 = w16.astype(NP_F8)
    Wr16neg = (w16 - W16.astype(np.float32)).astype(NP_F8)  # = -(W16 - 16w2t)
    sw = np.zeros((SCHUNKS * 128, 768), NP_F8)
    sw[0 : 3 * R : 3] = W16
    sw[1 : 3 * R : 3] = W16
    sw[2 : 3 * R : 3] = Wr16neg
    wconv8 = np.zeros((128, NT * N_DR * 256), NP_F8)
    for t in range(NT):
        for mi in range(N_DR):
            m = MSTART[t] + mi
            idx = t * N_DR + mi
            blk = sw[256 * m : 256 * (m + 1), 128 * t : 128 * (t + 1)]
            wconv8[:, idx * 256 : (idx + 1) * 256] = np.ascontiguousarray(
                blk.reshape(2, 128, 128).transpose(1, 0, 2).reshape(128, 256)
            )

    b1p = att_b1 + C_SELU * (att_w1 @ np.ones(K, np.float32))
    b2p = att_b2 + C_SELU * (att_w2 @ np.ones(U, np.float32))

    wmat = np.zeros((128, 388), np.float32)
    wbias = np.zeros((128, 6), np.float32)
    for q in range(QS):
        sl = slice(32 * q, 32 * q + 32)
        wmat[sl, 0:128][:, sl] = att_w1.T          # w1bd
        wmat[sl, 128:256][:, sl] = att_w2.T        # w2bd
        wmat[sl, 256 + q] = att_w3[0]              # w3bd
        wmat[q, 260 + 32 * q : 260 + 32 * q + 32] = 1.0  # bc4
        wbias[sl, 0] = conv_b + LN_LA
        wbias[sl, 1] = LAM * conv_b
        wbias[sl, 2] = b1p + LN_LA
        wbias[sl, 3] = LAM * b1p
        wbias[sl, 4] = b2p + LN_LA
        wbias[sl, 5] = LAM * b2p
    wmat16 = wmat.astype(NP_BF16)

    # ---------------- per-core inputs + bag bookkeeping
    xf = x.reshape(n_total, R)
    seg_pad = np.concatenate([seg, np.full(n_pad, N_BAGS, seg.dtype)])
    in_maps = []
    bagA = np.zeros((N_CORES, T, QS), np.int64)
    bagB = np.full((N_CORES, T, QS), -1, np.int64)
    npad_sub = np.zeros((N_CORES, T, QS), np.int64)
    for c in range(N_CORES):
        s0 = c * NPAD
        xt = np.zeros((R, NPAD), np.float32)
        real = min(NPAD, max(0, n_total - s0))
        if real > 0:
            xt[:, :real] = xf[s0 : s0 + real].T
        x8 = xt.astype(NP_F8)
        r8 = (xt - x8.astype(np.float32)).astype(NP_F8)
        sxr = np.zeros((SCHUNKS * 128, NPAD), NP_F8)
        sxr[0 : 3 * R : 3] = x8
        sxr[1 : 3 * R : 3] = r8
        sxr[2 : 3 * R : 3] = x8
        sx = np.ascontiguousarray(
            sxr.reshape(SCHUNKS, 128, NPAD).transpose(1, 0, 2).reshape(
                128, SCHUNKS * NPAD
            )
        )
        ids = seg_pad[s0 : s0 + NPAD].reshape(T, QS, FD)
        first = ids[:, :, 0]
        # real-instance last bag per subtile (pad slots hold N_BAGS)
        real_mask = ids < N_BAGS
        last_real = np.where(real_mask, ids, -1).max(axis=2)
        if ((last_real - first) > 1).any() and (last_real >= 0).all():
            raise ValueError("subtile spans >2 bags; unsupported input shape")
        bagA[c] = np.where(first < N_BAGS, first, -1)
        hasB = (last_real > first) & (first < N_BAGS)
        bagB[c] = np.where(hasB, last_real, -1)
        npad_sub[c] = (~real_mask).sum(axis=2)
        maskA = (ids == first[:, :, None]).astype(np.float32)
        maskA *= real_mask  # pad slots excluded even if first is pad
        # layout [QS, T*FD]
        maskp = np.ascontiguousarray(
            maskA.transpose(1, 0, 2).reshape(QS, T * FD)
        ).astype(NP_BF16)
        in_maps.append(
            {
                "sx": sx,
                "wconv": wconv8,
                "wmat": wmat16,
                "wbias": wbias,
                "maskp": maskp,
            }
        )

    nc = _get_program(NPAD)
    trace_mode = int(os.environ.get("DEEPRC_TRACE", "0"))
    kwargs = {}
    if trace_mode == 1:
        kwargs = dict(trace=True, trace_cores=[0])
    elif trace_mode >= 2:
        kwargs = dict(trace=True, trace_cores=list(range(N_CORES)), stitch_traces=True)
    res = run_bass_kernel_spmd(
        nc,
        in_maps,
        core_ids=list(range(N_CORES)),
        **kwargs,
    )
    LAST_RESULTS = res

    # ---------------- pad-instance constants (host, float64-exact path)
    # a zero input row gives er4 = 0 (conv of zeros, max of zeros), so the
    # device computes for each pad slot: e_pad = selu(conv_b)+LA etc.  The
    # unmasked totals z/pooled include those; subtract exactly here.
    if n_pad > 0:
        e_pad = np.where(
            conv_b > 0, LAM * conv_b, LA * np.exp(conv_b) - LA
        ) + LA  # selu(conv_b) + LA, shape [K]
        hh = np.where(
            att_w1 @ (e_pad + C_SELU) + att_b1 > 0,
            LAM * (att_w1 @ (e_pad + C_SELU) + att_b1),
            LA * np.exp(att_w1 @ (e_pad + C_SELU) + att_b1) - LA,
        )
        hh2 = np.where(
            att_w2 @ hh + att_b2 > 0,
            LAM * (att_w2 @ hh + att_b2),
            LA * np.exp(att_w2 @ hh + att_b2) - LA,
        )
        att_pad = float(att_w3[0] @ hh2)
        pz_pad = float(np.exp(att_pad))
        ppool_pad = pz_pad * e_pad  # [K]
    else:
        pz_pad = 0.0
        ppool_pad = np.zeros(K)

    # ---------------- exact host combine (float64)
    Z = np.zeros(N_BAGS, np.float64)
    P = np.zeros((N_BAGS, K), np.float64)
    for c in range(N_CORES):
        r = res.results[c]
        z = r["z_out"].astype(np.float64)            # [4, T]
        za = r["za_out"].astype(np.float64)          # [4, T]
        pooled = r["pooled_out"].astype(np.float64).reshape(QS, K, T)
        pooleda = r["pooleda_out"].astype(np.float64).reshape(QS, K, T)
        for j in range(T):
            for q in range(QS):
                bA = bagA[c, j, q]
                if bA < 0:
                    continue
                bB = bagB[c, j, q]
                if bB < 0:
                    Z[bA] += za[q, j]
                    P[bA] += pooleda[q, :, j]
                else:
                    Z[bA] += za[q, j]
                    P[bA] += pooleda[q, :, j]
                    npd = npad_sub[c, j, q]
                    Z[bB] += z[q, j] - za[q, j] - npd * pz_pad
                    P[bB] += (
                        pooled[q, :, j] - pooleda[q, :, j] - npd * ppool_pad
                    )

    out = np.zeros((N_BAGS, 1), np.float32)
    for b in range(N_BAGS):
        pooled_bag = P[b] / Z[b] + C_SELU
        out[b, 0] = np.float32(
            float(out_w.astype(np.float64)[0] @ pooled_bag) + float(out_b[0])
        )
    return out


# revision 27
# speedup vs baseline: 1.1617x; 1.0799x over previous
"""DeepRC segment-softmax attention pooling kernel for 8 Trainium2 NeuronCores.

Strategy v2 (balanced shard, zero collectives, bf16 datapath):
  - N=131072 sorted instances split evenly: core c gets rows
    [c*16384, (c+1)*16384) -- no padding (vs per-bag pad to 18432 before).
    Bag boundaries fall inside at most one 512-subtile per boundary; those
    subtiles are split on host: kernel emits BOTH bagA-masked stats (zA,
    pooledA) and unmasked totals (z, pooled); host assigns B-side = tot - A.
  - xt is pre-transposed to [(l,c)=736 rows, 16384 cols] bf16; one core
    macrotile (2048 cols) loads with 2 batched DMAs (vs 6 unbatched f32).
  - Conv1d(K=32,C=23,KS=9,L=32->24) stays a banded matmul in bf16: 20
    (t,s) blocks of [128x128]; PSUM [128, 6, 256] per half-subtile.
  - maxpool over l: DVE reduces blocks 0-2, Pool max-chains blocks 3-5,
    DVE pair-maxes into bf16; partition fold 128->32 via Act copies + DVE
    bf16 maxes (2x DVE mode).
  - SELU split: Act relu + Act exp + min/add scalar_tensor_tensor on Pool;
    selu constant -lam*alpha deferred to host biases/output.
  - Attention logits use m=0 (|att| ~ O(1), exp safe in f32); host combine
    is exact in float64, so no per-subtile max is needed at all.
  - Attention MLP runs block-diagonal bf16 [128,512] per macrotile.
"""

import os
import sys

for _p in (
    "/root/.axon_site",
    "/root/.axon_site/_ro/trn_rl_repo",
    "/root/.axon_site/_ro/pypackages",
    "/opt/trn_rl_repo",
):
    if os.path.isdir(_p) and _p not in sys.path:
        sys.path.append(_p)

import numpy as np

import concourse.bass as bass
import concourse.mybir as mybir
from concourse.tile import TileContext, ScopedClock
from concourse.bass_utils import run_bass_kernel_spmd

AF = mybir.ActivationFunctionType
OP = mybir.AluOpType
AX = mybir.AxisListType
F32 = mybir.dt.float32
BF16 = mybir.dt.bfloat16
F8E4 = mybir.dt.float8e4
PM = mybir.MatmulPerfMode
NP_BF16 = mybir.dt.np(mybir.dt.bfloat16)
NP_F8 = mybir.dt.np(mybir.dt.float8e4)

# ---------------------------------------------------------------- constants
N_BAGS = 8
N_CORES = 8
L, C, K, U, KS = 32, 23, 32, 32, 9
LO = L - KS + 1            # 24 output positions
R = L * C                  # 736 rows of xT
NT = 6                     # PSUM M blocks (each 4 l x 32 k)
FD = 512                   # instances per subtile
HFD = 256                  # half-subtile (PSUM bank sizing)
QS = 4                     # subtiles stacked per macrotile
MACRO = QS * FD            # 2048

LAM = 1.0507009873554805
ALPHA = 1.6732632423543772
LA = LAM * ALPHA
LN_LA = float(np.log(LA))
C_SELU = -LA               # deferred selu constant

# ------------------------------------------------------- walrus workarounds


def _patched_drain_and_barrier(self, tick_clock, wait_clock):
    # stock version puts every outstanding sem wait on one drain; this
    # walrus build allows a single sync wait per instruction.
    nc = self.nc
    drain_inst = nc.sync.drain()
    wait_clock.add_sem_waits(
        drain_inst.ins, ScopedClock({None: tick_clock.global_clock})
    )
    si = drain_inst.ins.sync_info
    waits = list(si.on_wait or []) if si is not None else []
    if len(waits) > 1:
        si.on_wait = waits[:1]
        for w in waits[1:]:
            extra = nc.sync.drain()
            esi = extra.ins.sync_info
            if esi is None:
                extra.ins.sync_info = mybir.SyncInfo(on_wait=[w], on_update=[])
            else:
                esi.on_wait = [w]
    nc.all_engine_barrier()
    assert self.sems is not None
    popped = nc._tile_sem_poison_stack.pop()
    assert popped is self._sem_poison
    nc.clear_and_free_semaphores(list(self.sems.allocated().values()))
    nc.all_engine_barrier()


TileContext._drain_and_barrier = _patched_drain_and_barrier

_WSPLIT_CTR = [0]


def _split_multi_waits(nc):
    # move extra sem waits onto same-engine NoOps inserted just before the
    # owning instruction (equivalent gating, one wait per instruction).
    for func in nc.m.functions:
        for blk in func.blocks:
            out = []
            changed = False
            for inst in blk.instructions:
                si = inst.sync_info
                if si is not None and si.on_wait is not None and len(si.on_wait) > 1:
                    waits = list(si.on_wait)
                    for w in waits[:-1]:
                        _WSPLIT_CTR[0] += 1
                        nop = mybir.InstNoOp(
                            name=f"I-wsplit-{_WSPLIT_CTR[0]}", ins=[], outs=[]
                        )
                        nop.engine = inst.engine
                        nop.sync_info = mybir.SyncInfo(on_wait=[w], on_update=[])
                        out.append(nop)
                    si.on_wait = [waits[-1]]
                    changed = True
                out.append(inst)
            if changed:
                blk.instructions[:] = out
    return nc


# ------------------------------------------------------------- conv blocks


def _conv_block_list():
    """Nonzero (t, s) blocks of the banded weight matrix, t-major."""
    blocks = []
    for t in range(NT):
        lo_row = 23 * (4 * t)            # first needed row: l' = 4t
        hi_row = 23 * (4 * t + 12) + 22  # last needed row: l' = 4t+12, c=22
        s_lo, s_hi = lo_row // 128, hi_row // 128
        for s in range(s_lo, min(s_hi, 5) + 1):
            blocks.append((t, s))
    return blocks


CONV_BLOCKS = _conv_block_list()          # 20 blocks
N_CB = len(CONV_BLOCKS)

# fp8 DoubleRow conv at 16x weight scale with W-only residual correction:
#   16*w2t ~= W16 - Wr16, W16 = fp8(16 w2t), Wr16 = fp8(W16 - 16 w2t)
#   16*y   ~= W16 @ x8 - Wr16 @ x8   (x fp8 noise averages out downstream)
# The moving operand is a single x8 region (736 rows, 6 chunks of 128);
# each out-block t runs two 2-chunk-pair accumulation passes (W16, -Wr16)
# over rows [92t, 92t+276).
SCHUNKS = 6
MSTART = [(92 * t) // 256 for t in range(NT)]    # first chunk-pair per t
N_DR = 2                       # chunk-pairs per pass per out-block


def _build_w2t(conv_w):
    w2t = np.zeros((768, 768), np.float32)
    for l in range(LO):
        for j in range(KS):
            lp = l + j
            # rows 23*lp .. +23 ; cols 32*l .. +32 ; value w[k, c, j]
            w2t[23 * lp : 23 * lp + 23, 32 * l : 32 * l + 32] = conv_w[:, :, j].T
    return w2t


# --------------------------------------------------------------- program


def _build_program(NPAD):
    T = NPAD // MACRO
    nc = bass.Bass()
    sx_d = nc.declare_dram_parameter("sx", [128, SCHUNKS * NPAD], F8E4, isOutput=False)
    wconv_d = nc.declare_dram_parameter("wconv", [128, NT * 2 * N_DR * 256], F8E4, isOutput=False)
    wmat_d = nc.declare_dram_parameter("wmat", [128, 388], BF16, isOutput=False)
    wbias_d = nc.declare_dram_parameter("wbias", [128, 6], F32, isOutput=False)
    mask_d = nc.declare_dram_parameter("maskp", [QS, T * FD], BF16, isOutput=False)
    z_out = nc.declare_dram_parameter("z_out", [QS, T], F32, isOutput=True)
    za_out = nc.declare_dram_parameter("za_out", [QS, T], F32, isOutput=True)
    pooled_out = nc.declare_dram_parameter("pooled_out", [128, T], F32, isOutput=True)
    pooleda_out = nc.declare_dram_parameter("pooleda_out", [128, T], F32, isOutput=True)

    with TileContext(nc) as tc:
        with (
            tc.tile_pool(name="wpool", bufs=1) as wpool,
            tc.tile_pool(name="xpool", bufs=2) as xpool,
            tc.tile_pool(name="spool", bufs=3) as spool,
            tc.tile_pool(name="cpsum", bufs=2, space="PSUM") as cpsum,
            tc.tile_pool(name="mpsum", bufs=2, space="PSUM") as mpsum,
        ):
            wsb = wpool.tile([128, NT * 2 * N_DR * 256], F8E4)
            nc.sync.dma_start(wsb[:], wconv_d[:])
            wmat = wpool.tile([128, 388], BF16)
            nc.sync.dma_start(wmat[:], wmat_d[:])
            wbias = wpool.tile([128, 6], F32)
            nc.sync.dma_start(wbias[:], wbias_d[:])
            mask_sb = wpool.tile([QS, T * FD], BF16)
            nc.sync.dma_start(mask_sb[:], mask_d[:])
            z_sb = wpool.tile([QS, T], F32)
            za_sb = wpool.tile([QS, T], F32)
            pooled_sb = wpool.tile([128, T], F32)
            pooleda_sb = wpool.tile([128, T], F32)

            w1bd = wmat[:, 0:128]
            w2bd = wmat[:, 128:256]
            w3bd = wmat[:, 256:260]
            bc4 = wmat[0:4, 260:388]
            be_exp = wbias[:, 0:1]
            be_relu = wbias[:, 1:2]
            bh1_exp = wbias[:, 2:3]
            bh1_relu = wbias[:, 3:4]
            bh2_exp = wbias[:, 4:5]
            bh2_relu = wbias[:, 5:6]

            for j in range(T):
                xts = xpool.tile([128, SCHUNKS, MACRO], F8E4, tag="xts")
                col0 = j * MACRO
                # partition-outer strided DMAs, one per subtile so the first
                # conv can start ~1us after the load begins (512B elem runs)
                for qq in range(QS):
                    nc.sync.dma_start(
                        xts[:, :, qq * FD : (qq + 1) * FD],
                        sx_d[:].rearrange("p (s f) -> p s f", s=SCHUNKS)[
                            :, :, col0 + qq * FD : col0 + (qq + 1) * FD
                        ],
                    )
                er4 = spool.tile([128, FD], BF16, tag="er4")
                for q in range(QS):
                    bm = spool.tile([128, FD], BF16, tag="bm")
                    for cc in range(2):
                        c0 = q * FD + cc * HFD
                        ps = cpsum.tile([128, NT, HFD], F32, tag="cps")
                        for t in range(NT):
                            for pi in range(2 * N_DR):   # (pass, mi) flat
                                m = MSTART[t] + (pi % N_DR)
                                idx = t * 2 * N_DR + pi
                                nc.tensor.matmul(
                                    ps[:, t, :],
                                    wsb[
                                        :, idx * 256 : (idx + 1) * 256
                                    ].rearrange("p (i m) -> p i m", i=2),
                                    xts[:, 2 * m : 2 * m + 2, c0 : c0 + HFD],
                                    start=(pi == 0),
                                    stop=(pi == 2 * N_DR - 1),
                                    perf_mode=PM.DoubleRow,
                                )
                        # max over the 6 l-blocks: GPSIMD cannot read PSUM,
                        # so Act exports blocks to SBUF bf16 and DVE merges
                        # at 2x; halves alternate the DVE/Act split to
                        # balance both engines.
                        bmo = bm[:, cc * HFD : (cc + 1) * HFD]
                        if cc == 0:
                            # DVE reduces blocks 0-1, Act copies 2-5
                            c4 = spool.tile([128, 4, HFD], BF16, tag="c4")
                            nc.scalar.copy(c4[:], ps[:, 2:6, :])
                            bmr = spool.tile([128, HFD], BF16, tag="bmr")
                            nc.vector.tensor_reduce(
                                bmr[:],
                                ps[:, 0:2, :].rearrange("p t f -> p f t"),
                                axis=AX.X, op=OP.max,
                            )
                            d2 = spool.tile([128, 2, HFD], BF16, tag="d2")
                            nc.vector.tensor_tensor(
                                d2[:], c4[:, 0:2, :], c4[:, 2:4, :], op=OP.max
                            )
                            d1 = spool.tile([128, HFD], BF16, tag="d1")
                            nc.vector.tensor_max(d1[:], d2[:, 0, :], d2[:, 1, :])
                            nc.vector.tensor_max(bmo, bmr[:], d1[:])
                        else:
                            # Act copies all 6 blocks, DVE merges the tree
                            c6 = spool.tile([128, 6, HFD], BF16, tag="c6")
                            nc.scalar.copy(c6[:], ps[:])
                            e3 = spool.tile([128, 3, HFD], BF16, tag="e3")
                            nc.vector.tensor_tensor(
                                e3[:], c6[:, 0:3, :], c6[:, 3:6, :], op=OP.max
                            )
                            e1 = spool.tile([128, HFD], BF16, tag="e1")
                            nc.vector.tensor_max(e1[:], e3[:, 0, :], e3[:, 1, :])
                            nc.vector.tensor_max(bmo, e1[:], e3[:, 2, :])
                    # partition fold 128 -> 32 (max over 4 l-residue groups)
                    t64 = spool.tile([64, FD], BF16, tag="t64")
                    nc.gpsimd.tensor_copy(t64[:], bm[64:128, :])
                    f1 = spool.tile([64, FD], BF16, tag="f1")
                    nc.vector.tensor_max(f1[:], bm[0:64, :], t64[:])
                    t32 = spool.tile([32, FD], BF16, tag="t32")
                    nc.gpsimd.tensor_copy(t32[:], f1[32:64, :])
                    nc.vector.tensor_max(
                        er4[32 * q : 32 * q + 32, :], f1[0:32, :], t32[:]
                    )

                # ---- selu(er4/16 + conv_b): the fp8 conv runs at 16x scale,
                # folded into the activation scale (maxpool commutes)
                t_relu = spool.tile([128, FD], F32, tag="t_relu")
                nc.scalar.activation(
                    t_relu[:], er4[:], AF.Relu, bias=be_relu, scale=LAM / 16.0
                )
                v_exp = spool.tile([128, FD], F32, tag="v_exp")
                nc.scalar.activation(
                    v_exp[:], er4[:], AF.Exp, bias=be_exp, scale=1.0 / 16.0
                )
                e4 = spool.tile([128, FD], BF16, tag="e4")
                nc.vector.scalar_tensor_tensor(
                    e4[:], v_exp[:], LA, t_relu[:], op0=OP.min, op1=OP.add
                )
                # ---- MLP layer 1
                ps1 = mpsum.tile([128, FD], F32, tag="mlp")
                nc.tensor.matmul(ps1[:], w1bd, e4[:])
                t1 = spool.tile([128, FD], F32, tag="t1")
                nc.scalar.activation(t1[:], ps1[:], AF.Relu, bias=bh1_relu, scale=LAM)
                v1 = spool.tile([128, FD], F32, tag="v1")
                nc.scalar.activation(v1[:], ps1[:], AF.Exp, bias=bh1_exp, scale=1.0)
                h1 = spool.tile([128, FD], BF16, tag="h1")
                nc.vector.scalar_tensor_tensor(
                    h1[:], v1[:], LA, t1[:], op0=OP.min, op1=OP.add
                )
                # ---- MLP layer 2
                ps2 = mpsum.tile([128, FD], F32, tag="mlp")
                nc.tensor.matmul(ps2[:], w2bd, h1[:])
                t2 = spool.tile([128, FD], F32, tag="t2")
                nc.scalar.activation(t2[:], ps2[:], AF.Relu, bias=bh2_relu, scale=LAM)
                v2 = spool.tile([128, FD], F32, tag="v2")
                nc.scalar.activation(v2[:], ps2[:], AF.Exp, bias=bh2_exp, scale=1.0)
                h2 = spool.tile([128, FD], BF16, tag="h2")
                nc.vector.scalar_tensor_tensor(
                    h2[:], v2[:], LA, t2[:], op0=OP.min, op1=OP.add
                )
                # ---- attention logits (b3 cancels in softmax; m == 0)
                psa = mpsum.tile([4, FD], F32, tag="mlp")
                nc.tensor.matmul(psa[:], w3bd, h2[:])
                pexp = spool.tile([4, FD], BF16, tag="pexp")
                nc.scalar.activation(
                    pexp[:], psa[:], AF.Exp, bias=0.0, scale=1.0,
                    accum_out=z_sb[:, j : j + 1],
                )
                p4a = spool.tile([4, FD], BF16, tag="p4a")
                nc.vector.scalar_tensor_tensor(
                    p4a[:], pexp[:], 1.0, mask_sb[:, j * FD : (j + 1) * FD],
                    op0=OP.mult, op1=OP.mult, accum_out=za_sb[:, j : j + 1],
                )
                # ---- pooled(+A) += e4 * broadcast(p) per subtile group
                psbT = mpsum.tile([128, FD], F32, tag="mlp")
                nc.tensor.matmul(psbT[:], bc4, pexp[:])
                psbA = mpsum.tile([128, FD], F32, tag="mlp")
                nc.tensor.matmul(psbA[:], bc4, p4a[:])
                weT = spool.tile([128, FD], F32, tag="weT")
                nc.vector.scalar_tensor_tensor(
                    weT[:], e4[:], 1.0, psbT[:],
                    op0=OP.mult, op1=OP.mult, accum_out=pooled_sb[:, j : j + 1],
                )
                weA = spool.tile([128, FD], F32, tag="weA")
                nc.vector.scalar_tensor_tensor(
                    weA[:], e4[:], 1.0, psbA[:],
                    op0=OP.mult, op1=OP.mult, accum_out=pooleda_sb[:, j : j + 1],
                )

            nc.sync.dma_start(z_out[:], z_sb[:])
            nc.sync.dma_start(za_out[:], za_sb[:])
            nc.sync.dma_start(pooled_out[:], pooled_sb[:])
            nc.sync.dma_start(pooleda_out[:], pooleda_sb[:])

    _split_multi_waits(nc)
    return nc


_PROGRAM_CACHE = {}
LAST_RESULTS = None  # set by kernel(); test.py reads trace/exec info


def _get_program(NPAD):
    if NPAD not in _PROGRAM_CACHE:
        _PROGRAM_CACHE[NPAD] = _build_program(NPAD)
    return _PROGRAM_CACHE[NPAD]


# ----------------------------------------------------------------- kernel


def kernel(
    inputs,
    segment_ids,
    conv_w,
    conv_b,
    att_w1,
    att_b1,
    att_w2,
    att_b2,
    att_w3,
    att_b3,
    out_w,
    out_b,
):
    global LAST_RESULTS
    x = np.asarray(inputs, np.float32)
    seg = np.asarray(segment_ids)
    conv_w = np.asarray(conv_w, np.float32)
    conv_b = np.asarray(conv_b, np.float32)
    att_w1 = np.asarray(att_w1, np.float32)
    att_b1 = np.asarray(att_b1, np.float32)
    att_w2 = np.asarray(att_w2, np.float32)
    att_b2 = np.asarray(att_b2, np.float32)
    att_w3 = np.asarray(att_w3, np.float32)
    att_b3 = np.asarray(att_b3, np.float32)
    out_w = np.asarray(out_w, np.float32)
    out_b = np.asarray(out_b, np.float32)

    n_total = x.shape[0]
    NPAD = -(-n_total // (N_CORES * MACRO)) * MACRO   # per-core cols
    T = NPAD // MACRO
    n_padded = N_CORES * NPAD
    n_pad = n_padded - n_total

    # ---------------- weights (shared by all cores)
    # fp8 DoubleRow conv at 16x scale, W-only residual (see constants above)
    w2t = _build_w2t(conv_w)
    w16 = 16.0 * w2t[:R]                               # [736, 768]
    W16 = w16.astype(NP_F8)
    Wr16neg = (w16 - W16.astype(np.float32)).astype(NP_F8)  # = -(W16 - 16w2t)
    passes = []
    for Wp in (W16, Wr16neg):
        sw = np.zeros((SCHUNKS * 128, 768), NP_F8)
        sw[:R] = Wp
        passes.append(sw)
    wconv8 = np.zeros((128, NT * 2 * N_DR * 256), NP_F8)
    for t in range(NT):
        for pi in range(2 * N_DR):
            m = MSTART[t] + (pi % N_DR)
            sw = passes[pi // N_DR]
            idx = t * 2 * N_DR + pi
            blk = sw[256 * m : 256 * (m + 1), 128 * t : 128 * (t + 1)]
            wconv8[:, idx * 256 : (idx + 1) * 256] = np.ascontiguousarray(
                blk.reshape(2, 128, 128).transpose(1, 0, 2).reshape(128, 256)
            )

    b1p = att_b1 + C_SELU * (att_w1 @ np.ones(K, np.float32))
    b2p = att_b2 + C_SELU * (att_w2 @ np.ones(U, np.float32))

    wmat = np.zeros((128, 388), np.float32)
    wbias = np.zeros((128, 6), np.float32)
    for q in range(QS):
        sl = slice(32 * q, 32 * q + 32)
        wmat[sl, 0:128][:, sl] = att_w1.T          # w1bd
        wmat[sl, 128:256][:, sl] = att_w2.T        # w2bd
        wmat[sl, 256 + q] = att_w3[0]              # w3bd
        wmat[q, 260 + 32 * q : 260 + 32 * q + 32] = 1.0  # bc4
        wbias[sl, 0] = conv_b + LN_LA
        wbias[sl, 1] = LAM * conv_b
        wbias[sl, 2] = b1p + LN_LA
        wbias[sl, 3] = LAM * b1p
        wbias[sl, 4] = b2p + LN_LA
        wbias[sl, 5] = LAM * b2p
    wmat16 = wmat.astype(NP_BF16)

    # ---------------- per-core inputs + bag bookkeeping
    xf = x.reshape(n_total, R)
    seg_pad = np.concatenate([seg, np.full(n_pad, N_BAGS, seg.dtype)])
    in_maps = []
    bagA = np.zeros((N_CORES, T, QS), np.int64)
    bagB = np.full((N_CORES, T, QS), -1, np.int64)
    npad_sub = np.zeros((N_CORES, T, QS), np.int64)
    for c in range(N_CORES):
        s0 = c * NPAD
        xt = np.zeros((R, NPAD), np.float32)
        real = min(NPAD, max(0, n_total - s0))
        if real > 0:
            xt[:, :real] = xf[s0 : s0 + real].T
        sxr = np.zeros((SCHUNKS * 128, NPAD), NP_F8)
        sxr[:R] = xt.astype(NP_F8)
        sx = np.ascontiguousarray(
            sxr.reshape(SCHUNKS, 128, NPAD).transpose(1, 0, 2).reshape(
                128, SCHUNKS * NPAD
            )
        )
        ids = seg_pad[s0 : s0 + NPAD].reshape(T, QS, FD)
        first = ids[:, :, 0]
        # real-instance last bag per subtile (pad slots hold N_BAGS)
        real_mask = ids < N_BAGS
        last_real = np.where(real_mask, ids, -1).max(axis=2)
        if ((last_real - first) > 1).any() and (last_real >= 0).all():
            raise ValueError("subtile spans >2 bags; unsupported input shape")
        bagA[c] = np.where(first < N_BAGS, first, -1)
        hasB = (last_real > first) & (first < N_BAGS)
        bagB[c] = np.where(hasB, last_real, -1)
        npad_sub[c] = (~real_mask).sum(axis=2)
        maskA = (ids == first[:, :, None]).astype(np.float32)
        maskA *= real_mask  # pad slots excluded even if first is pad
        # layout [QS, T*FD]
        maskp = np.ascontiguousarray(
            maskA.transpose(1, 0, 2).reshape(QS, T * FD)
        ).astype(NP_BF16)
        in_maps.append(
            {
                "sx": sx,
                "wconv": wconv8,
                "wmat": wmat16,
                "wbias": wbias,
                "maskp": maskp,
            }
        )

    nc = _get_program(NPAD)
    trace_mode = int(os.environ.get("DEEPRC_TRACE", "0"))
    kwargs = {}
    if trace_mode == 1:
        kwargs = dict(trace=True, trace_cores=[0])
    elif trace_mode >= 2:
        kwargs = dict(trace=True, trace_cores=list(range(N_CORES)), stitch_traces=True)
    res = run_bass_kernel_spmd(
        nc,
        in_maps,
        core_ids=list(range(N_CORES)),
        **kwargs,
    )
    LAST_RESULTS = res

    # ---------------- pad-instance constants (host, float64-exact path)
    # a zero input row gives er4 = 0 (conv of zeros, max of zeros), so the
    # device computes for each pad slot: e_pad = selu(conv_b)+LA etc.  The
    # unmasked totals z/pooled include those; subtract exactly here.
    if n_pad > 0:
        e_pad = np.where(
            conv_b > 0, LAM * conv_b, LA * np.exp(conv_b) - LA
        ) + LA  # selu(conv_b) + LA, shape [K]
        hh = np.where(
            att_w1 @ (e_pad + C_SELU) + att_b1 > 0,
            LAM * (att_w1 @ (e_pad + C_SELU) + att_b1),
            LA * np.exp(att_w1 @ (e_pad + C_SELU) + att_b1) - LA,
        )
        hh2 = np.where(
            att_w2 @ hh + att_b2 > 0,
            LAM * (att_w2 @ hh + att_b2),
            LA * np.exp(att_w2 @ hh + att_b2) - LA,
        )
        att_pad = float(att_w3[0] @ hh2)
        pz_pad = float(np.exp(att_pad))
        ppool_pad = pz_pad * e_pad  # [K]
    else:
        pz_pad = 0.0
        ppool_pad = np.zeros(K)

    # ---------------- exact host combine (float64)
    Z = np.zeros(N_BAGS, np.float64)
    P = np.zeros((N_BAGS, K), np.float64)
    for c in range(N_CORES):
        r = res.results[c]
        z = r["z_out"].astype(np.float64)            # [4, T]
        za = r["za_out"].astype(np.float64)          # [4, T]
        pooled = r["pooled_out"].astype(np.float64).reshape(QS, K, T)
        pooleda = r["pooleda_out"].astype(np.float64).reshape(QS, K, T)
        for j in range(T):
            for q in range(QS):
                bA = bagA[c, j, q]
                if bA < 0:
                    continue
                bB = bagB[c, j, q]
                if bB < 0:
                    Z[bA] += za[q, j]
                    P[bA] += pooleda[q, :, j]
                else:
                    Z[bA] += za[q, j]
                    P[bA] += pooleda[q, :, j]
                    npd = npad_sub[c, j, q]
                    Z[bB] += z[q, j] - za[q, j] - npd * pz_pad
                    P[bB] += (
                        pooled[q, :, j] - pooleda[q, :, j] - npd * ppool_pad
                    )

    out = np.zeros((N_BAGS, 1), np.float32)
    for b in range(N_BAGS):
        pooled_bag = P[b] / Z[b] + C_SELU
        out[b, 0] = np.float32(
            float(out_w.astype(np.float64)[0] @ pooled_bag) + float(out_b[0])
        )
    return out


# revision 36
# speedup vs baseline: 1.2185x; 1.0488x over previous
"""DeepRC segment-softmax attention pooling kernel for 8 Trainium2 NeuronCores.

Strategy v2 (balanced shard, zero collectives, bf16 datapath):
  - N=131072 sorted instances split evenly: core c gets rows
    [c*16384, (c+1)*16384) -- no padding (vs per-bag pad to 18432 before).
    Bag boundaries fall inside at most one 512-subtile per boundary; those
    subtiles are split on host: kernel emits BOTH bagA-masked stats (zA,
    pooledA) and unmasked totals (z, pooled); host assigns B-side = tot - A.
  - xt is pre-transposed to [(l,c)=736 rows, 16384 cols] bf16; one core
    macrotile (2048 cols) loads with 2 batched DMAs (vs 6 unbatched f32).
  - Conv1d(K=32,C=23,KS=9,L=32->24) stays a banded matmul in bf16: 20
    (t,s) blocks of [128x128]; PSUM [128, 6, 256] per half-subtile.
  - maxpool over l: DVE reduces blocks 0-2, Pool max-chains blocks 3-5,
    DVE pair-maxes into bf16; partition fold 128->32 via Act copies + DVE
    bf16 maxes (2x DVE mode).
  - SELU split: Act relu + Act exp + min/add scalar_tensor_tensor on Pool;
    selu constant -lam*alpha deferred to host biases/output.
  - Attention logits use m=0 (|att| ~ O(1), exp safe in f32); host combine
    is exact in float64, so no per-subtile max is needed at all.
  - Attention MLP runs block-diagonal bf16 [128,512] per macrotile.
"""

import os
import sys

for _p in (
    "/root/.axon_site",
    "/root/.axon_site/_ro/trn_rl_repo",
    "/root/.axon_site/_ro/pypackages",
    "/opt/trn_rl_repo",
):
    if os.path.isdir(_p) and _p not in sys.path:
        sys.path.append(_p)

import numpy as np

import concourse.bass as bass
import concourse.mybir as mybir
from concourse.tile import TileContext, ScopedClock
from concourse.bass_utils import run_bass_kernel_spmd

AF = mybir.ActivationFunctionType
OP = mybir.AluOpType
AX = mybir.AxisListType
F32 = mybir.dt.float32
BF16 = mybir.dt.bfloat16
F8E4 = mybir.dt.float8e4
PM = mybir.MatmulPerfMode
NP_BF16 = mybir.dt.np(mybir.dt.bfloat16)
NP_F8 = mybir.dt.np(mybir.dt.float8e4)

# ---------------------------------------------------------------- constants
N_BAGS = 8
N_CORES = 8
L, C, K, U, KS = 32, 23, 32, 32, 9
LO = L - KS + 1            # 24 output positions
R = L * C                  # 736 rows of xT
NT = 6                     # PSUM M blocks (each 4 l x 32 k)
FD = 512                   # instances per subtile
HFD = 256                  # half-subtile (PSUM bank sizing)
QS = 4                     # subtiles stacked per macrotile
MACRO = QS * FD            # 2048

LAM = 1.0507009873554805
ALPHA = 1.6732632423543772
LA = LAM * ALPHA
LN_LA = float(np.log(LA))
C_SELU = -LA               # deferred selu constant

# ------------------------------------------------------- walrus workarounds


def _patched_drain_and_barrier(self, tick_clock, wait_clock):
    # stock version puts every outstanding sem wait on one drain; this
    # walrus build allows a single sync wait per instruction.
    nc = self.nc
    drain_inst = nc.sync.drain()
    wait_clock.add_sem_waits(
        drain_inst.ins, ScopedClock({None: tick_clock.global_clock})
    )
    si = drain_inst.ins.sync_info
    waits = list(si.on_wait or []) if si is not None else []
    if len(waits) > 1:
        si.on_wait = waits[:1]
        for w in waits[1:]:
            extra = nc.sync.drain()
            esi = extra.ins.sync_info
            if esi is None:
                extra.ins.sync_info = mybir.SyncInfo(on_wait=[w], on_update=[])
            else:
                esi.on_wait = [w]
    nc.all_engine_barrier()
    assert self.sems is not None
    popped = nc._tile_sem_poison_stack.pop()
    assert popped is self._sem_poison
    nc.clear_and_free_semaphores(list(self.sems.allocated().values()))
    nc.all_engine_barrier()


TileContext._drain_and_barrier = _patched_drain_and_barrier

_WSPLIT_CTR = [0]


def _split_multi_waits(nc):
    # move extra sem waits onto same-engine NoOps inserted just before the
    # owning instruction (equivalent gating, one wait per instruction).
    for func in nc.m.functions:
        for blk in func.blocks:
            out = []
            changed = False
            for inst in blk.instructions:
                si = inst.sync_info
                if si is not None and si.on_wait is not None and len(si.on_wait) > 1:
                    waits = list(si.on_wait)
                    for w in waits[:-1]:
                        _WSPLIT_CTR[0] += 1
                        nop = mybir.InstNoOp(
                            name=f"I-wsplit-{_WSPLIT_CTR[0]}", ins=[], outs=[]
                        )
                        nop.engine = inst.engine
                        nop.sync_info = mybir.SyncInfo(on_wait=[w], on_update=[])
                        out.append(nop)
                    si.on_wait = [waits[-1]]
                    changed = True
                out.append(inst)
            if changed:
                blk.instructions[:] = out
    return nc


# ------------------------------------------------------------- conv blocks


def _conv_block_list():
    """Nonzero (t, s) blocks of the banded weight matrix, t-major."""
    blocks = []
    for t in range(NT):
        lo_row = 23 * (4 * t)            # first needed row: l' = 4t
        hi_row = 23 * (4 * t + 12) + 22  # last needed row: l' = 4t+12, c=22
        s_lo, s_hi = lo_row // 128, hi_row // 128
        for s in range(s_lo, min(s_hi, 5) + 1):
            blocks.append((t, s))
    return blocks


CONV_BLOCKS = _conv_block_list()          # 20 blocks
N_CB = len(CONV_BLOCKS)

# fp8 DoubleRow conv at 16x weight scale with W-only residual correction:
#   16*w2t ~= W16 - Wr16, W16 = fp8(16 w2t), Wr16 = fp8(W16 - 16 w2t)
#   16*y   ~= W16 @ x8 - Wr16 @ x8   (x fp8 noise averages out downstream)
# The moving operand is a single x8 region (736 rows, 6 chunks of 128);
# each out-block t runs two 2-chunk-pair accumulation passes (W16, -Wr16)
# over rows [92t, 92t+276).
SCHUNKS = 6
MSTART = [(92 * t) // 256 for t in range(NT)]    # first chunk-pair per t
N_DR = 2                       # chunk-pairs per pass per out-block


def _build_w2t(conv_w):
    w2t = np.zeros((768, 768), np.float32)
    for l in range(LO):
        for j in range(KS):
            lp = l + j
            # rows 23*lp .. +23 ; cols 32*l .. +32 ; value w[k, c, j]
            w2t[23 * lp : 23 * lp + 23, 32 * l : 32 * l + 32] = conv_w[:, :, j].T
    return w2t


# --------------------------------------------------------------- program


def _build_program(NPAD):
    T = NPAD // MACRO
    nc = bass.Bass()
    sx_d = nc.declare_dram_parameter("sx", [128, SCHUNKS * NPAD], F8E4, isOutput=False)
    wconv_d = nc.declare_dram_parameter("wconv", [128, NT * 2 * N_DR * 256], F8E4, isOutput=False)
    wmat_d = nc.declare_dram_parameter("wmat", [128, 388], BF16, isOutput=False)
    wbias_d = nc.declare_dram_parameter("wbias", [128, 6], F32, isOutput=False)
    mask_d = nc.declare_dram_parameter("maskp", [QS, T * FD], BF16, isOutput=False)
    # single combined output: cols [0,T)=pooled, [T,2T)=pooledA,
    # cols [2T,3T) rows 0-3 = z, cols [3T,4T) rows 0-3 = zA
    out_d = nc.declare_dram_parameter("outs", [128, 4 * T], F32, isOutput=True)

    with TileContext(nc) as tc:
        with (
            tc.tile_pool(name="wpool", bufs=1) as wpool,
            tc.tile_pool(name="xpool", bufs=2) as xpool,
            tc.tile_pool(name="spool", bufs=3) as spool,
            tc.tile_pool(name="cpsum", bufs=2, space="PSUM") as cpsum,
            tc.tile_pool(name="mpsum", bufs=2, space="PSUM") as mpsum,
        ):
            # PE warm-up: the HAM clock gate needs ~3.4us of sustained PE
            # activity to release 2.4GHz; burn dummy matmuls on scratch data
            # while the first input DMAs are in flight.
            scratch = wpool.tile([128, 640], BF16)
            nc.gpsimd.memset(scratch[:], 0.0)
            for _ in range(6):
                wps = mpsum.tile([128, FD], F32, tag="mlp")
                nc.tensor.matmul(wps[:], scratch[:, 0:128], scratch[:, 128:640])

            wsb = wpool.tile([128, NT * 2 * N_DR * 256], F8E4)
            nc.sync.dma_start(wsb[:], wconv_d[:])
            wmat = wpool.tile([128, 388], BF16)
            nc.sync.dma_start(wmat[:], wmat_d[:])
            wbias = wpool.tile([128, 6], F32)
            nc.sync.dma_start(wbias[:], wbias_d[:])
            mask_sb = wpool.tile([QS, T * FD], BF16)
            nc.sync.dma_start(mask_sb[:], mask_d[:])
            outs_sb = wpool.tile([128, 4 * T], F32)
            nc.gpsimd.memset(outs_sb[:], 0.0)
            pooled_sb = outs_sb[:, 0:T]
            pooleda_sb = outs_sb[:, T : 2 * T]
            z_sb = outs_sb[0:QS, 2 * T : 3 * T]
            za_sb = outs_sb[0:QS, 3 * T : 4 * T]

            w1bd = wmat[:, 0:128]
            w2bd = wmat[:, 128:256]
            w3bd = wmat[:, 256:260]
            bc4 = wmat[0:4, 260:388]
            be_exp = wbias[:, 0:1]
            be_relu = wbias[:, 1:2]
            bh1_exp = wbias[:, 2:3]
            bh1_relu = wbias[:, 3:4]
            bh2_exp = wbias[:, 4:5]
            bh2_relu = wbias[:, 5:6]

            for j in range(T):
                xts = xpool.tile([128, SCHUNKS, MACRO], F8E4, tag="xts")
                col0 = j * MACRO
                # partition-outer strided DMAs, one per subtile so the first
                # conv can start ~1us after the load begins (512B elem runs)
                for qq in range(QS):
                    nc.sync.dma_start(
                        xts[:, :, qq * FD : (qq + 1) * FD],
                        sx_d[:].rearrange("p (s f) -> p s f", s=SCHUNKS)[
                            :, :, col0 + qq * FD : col0 + (qq + 1) * FD
                        ],
                    )
                er4 = spool.tile([128, FD], BF16, tag="er4")
                f1s = []
                for q in range(QS):
                    bm = spool.tile([128, FD], BF16, tag="bm")
                    for cc in range(2):
                        c0 = q * FD + cc * HFD
                        ps = cpsum.tile([128, NT, HFD], F32, tag="cps")
                        for t in range(NT):
                            for pi in range(2 * N_DR):   # (pass, mi) flat
                                m = MSTART[t] + (pi % N_DR)
                                idx = t * 2 * N_DR + pi
                                nc.tensor.matmul(
                                    ps[:, t, :],
                                    wsb[
                                        :, idx * 256 : (idx + 1) * 256
                                    ].rearrange("p (i m) -> p i m", i=2),
                                    xts[:, 2 * m : 2 * m + 2, c0 : c0 + HFD],
                                    start=(pi == 0),
                                    stop=(pi == 2 * N_DR - 1),
                                    perf_mode=PM.DoubleRow,
                                )
                        # max over the 6 l-blocks: GPSIMD cannot read PSUM,
                        # so Act exports blocks to SBUF bf16 and DVE merges
                        # at 2x; halves alternate the DVE/Act split to
                        # balance both engines.
                        bmo = bm[:, cc * HFD : (cc + 1) * HFD]
                        if cc == 0:
                            # DVE reduces blocks 0-1, Act copies 2-5
                            c4 = spool.tile([128, 4, HFD], BF16, tag="c4")
                            nc.scalar.copy(c4[:], ps[:, 2:6, :])
                            bmr = spool.tile([128, HFD], BF16, tag="bmr")
                            nc.vector.tensor_reduce(
                                bmr[:],
                                ps[:, 0:2, :].rearrange("p t f -> p f t"),
                                axis=AX.X, op=OP.max,
                            )
                            d2 = spool.tile([128, 2, HFD], BF16, tag="d2")
                            nc.vector.tensor_tensor(
                                d2[:], c4[:, 0:2, :], c4[:, 2:4, :], op=OP.max
                            )
                            d1 = spool.tile([128, HFD], BF16, tag="d1")
                            nc.vector.tensor_max(d1[:], d2[:, 0, :], d2[:, 1, :])
                            nc.vector.tensor_max(bmo, bmr[:], d1[:])
                        else:
                            # Act copies all 6 blocks, DVE merges the tree
                            c6 = spool.tile([128, 6, HFD], BF16, tag="c6")
                            nc.scalar.copy(c6[:], ps[:])
                            e3 = spool.tile([128, 3, HFD], BF16, tag="e3")
                            nc.vector.tensor_tensor(
                                e3[:], c6[:, 0:3, :], c6[:, 3:6, :], op=OP.max
                            )
                            e1 = spool.tile([128, HFD], BF16, tag="e1")
                            nc.vector.tensor_max(e1[:], e3[:, 0, :], e3[:, 1, :])
                            nc.vector.tensor_max(bmo, e1[:], e3[:, 2, :])
                    # partition fold stage 1: 128 -> 64
                    t64 = spool.tile([64, FD], BF16, tag="t64")
                    nc.gpsimd.tensor_copy(t64[:], bm[64:128, :])
                    f1 = spool.tile([64, FD], BF16, tag=f"f1{q}")
                    nc.vector.tensor_max(f1[:], bm[0:64, :], t64[:])
                    f1s.append(f1)
                # fold stage 2 deferred: the Pool copies of subtile q overlap
                # DVE merge work of subtile q+1 instead of serializing
                for q, f1 in enumerate(f1s):
                    t32 = spool.tile([32, FD], BF16, tag=f"t32{q % 2}")
                    nc.gpsimd.tensor_copy(t32[:], f1[32:64, :])
                    nc.vector.tensor_max(
                        er4[32 * q : 32 * q + 32, :], f1[0:32, :], t32[:]
                    )

                # ---- selu(er4/16 + conv_b): the fp8 conv runs at 16x scale,
                # folded into the activation scale (maxpool commutes)
                t_relu = spool.tile([128, FD], F32, tag="t_relu")
                nc.scalar.activation(
                    t_relu[:], er4[:], AF.Relu, bias=be_relu, scale=LAM / 16.0
                )
                v_exp = spool.tile([128, FD], F32, tag="v_exp")
                nc.scalar.activation(
                    v_exp[:], er4[:], AF.Exp, bias=be_exp, scale=1.0 / 16.0
                )
                e4 = spool.tile([128, FD], BF16, tag="e4")
                nc.vector.scalar_tensor_tensor(
                    e4[:], v_exp[:], LA, t_relu[:], op0=OP.min, op1=OP.add
                )
                # ---- MLP layer 1
                ps1 = mpsum.tile([128, FD], F32, tag="mlp")
                nc.tensor.matmul(ps1[:], w1bd, e4[:])
                t1 = spool.tile([128, FD], F32, tag="t1")
                nc.scalar.activation(t1[:], ps1[:], AF.Relu, bias=bh1_relu, scale=LAM)
                v1 = spool.tile([128, FD], F32, tag="v1")
                nc.scalar.activation(v1[:], ps1[:], AF.Exp, bias=bh1_exp, scale=1.0)
                h1 = spool.tile([128, FD], BF16, tag="h1")
                nc.vector.scalar_tensor_tensor(
                    h1[:], v1[:], LA, t1[:], op0=OP.min, op1=OP.add
                )
                # ---- MLP layer 2
                ps2 = mpsum.tile([128, FD], F32, tag="mlp")
                nc.tensor.matmul(ps2[:], w2bd, h1[:])
                t2 = spool.tile([128, FD], F32, tag="t2")
                nc.scalar.activation(t2[:], ps2[:], AF.Relu, bias=bh2_relu, scale=LAM)
                v2 = spool.tile([128, FD], F32, tag="v2")
                nc.scalar.activation(v2[:], ps2[:], AF.Exp, bias=bh2_exp, scale=1.0)
                h2 = spool.tile([128, FD], BF16, tag="h2")
                nc.vector.scalar_tensor_tensor(
                    h2[:], v2[:], LA, t2[:], op0=OP.min, op1=OP.add
                )
                # ---- attention logits (b3 cancels in softmax; m == 0)
                psa = mpsum.tile([4, FD], F32, tag="mlp")
                nc.tensor.matmul(psa[:], w3bd, h2[:])
                pexp = spool.tile([4, FD], BF16, tag="pexp")
                nc.scalar.activation(
                    pexp[:], psa[:], AF.Exp, bias=0.0, scale=1.0,
                    accum_out=z_sb[:, j : j + 1],
                )
                p4a = spool.tile([4, FD], BF16, tag="p4a")
                nc.vector.scalar_tensor_tensor(
                    p4a[:], pexp[:], 1.0, mask_sb[:, j * FD : (j + 1) * FD],
                    op0=OP.mult, op1=OP.mult, accum_out=za_sb[:, j : j + 1],
                )
                # ---- pooled(+A) += e4 * broadcast(p) per subtile group
                psbT = mpsum.tile([128, FD], F32, tag="mlp")
                nc.tensor.matmul(psbT[:], bc4, pexp[:])
                psbA = mpsum.tile([128, FD], F32, tag="mlp")
                nc.tensor.matmul(psbA[:], bc4, p4a[:])
                weT = spool.tile([128, FD], F32, tag="weT")
                nc.vector.scalar_tensor_tensor(
                    weT[:], e4[:], 1.0, psbT[:],
                    op0=OP.mult, op1=OP.mult, accum_out=pooled_sb[:, j : j + 1],
                )
                weA = spool.tile([128, FD], F32, tag="weA")
                nc.vector.scalar_tensor_tensor(
                    weA[:], e4[:], 1.0, psbA[:],
                    op0=OP.mult, op1=OP.mult, accum_out=pooleda_sb[:, j : j + 1],
                )

            nc.sync.dma_start(out_d[:], outs_sb[:])

    _split_multi_waits(nc)
    return nc


_PROGRAM_CACHE = {}
LAST_RESULTS = None  # set by kernel(); test.py reads trace/exec info


def _get_program(NPAD):
    if NPAD not in _PROGRAM_CACHE:
        _PROGRAM_CACHE[NPAD] = _build_program(NPAD)
    return _PROGRAM_CACHE[NPAD]


# ----------------------------------------------------------------- kernel


def kernel(
    inputs,
    segment_ids,
    conv_w,
    conv_b,
    att_w1,
    att_b1,
    att_w2,
    att_b2,
    att_w3,
    att_b3,
    out_w,
    out_b,
):
    global LAST_RESULTS
    x = np.asarray(inputs, np.float32)
    seg = np.asarray(segment_ids)
    conv_w = np.asarray(conv_w, np.float32)
    conv_b = np.asarray(conv_b, np.float32)
    att_w1 = np.asarray(att_w1, np.float32)
    att_b1 = np.asarray(att_b1, np.float32)
    att_w2 = np.asarray(att_w2, np.float32)
    att_b2 = np.asarray(att_b2, np.float32)
    att_w3 = np.asarray(att_w3, np.float32)
    att_b3 = np.asarray(att_b3, np.float32)
    out_w = np.asarray(out_w, np.float32)
    out_b = np.asarray(out_b, np.float32)

    n_total = x.shape[0]
    NPAD = -(-n_total // (N_CORES * MACRO)) * MACRO   # per-core cols
    T = NPAD // MACRO
    n_padded = N_CORES * NPAD
    n_pad = n_padded - n_total

    # ---------------- weights (shared by all cores)
    # fp8 DoubleRow conv at 16x scale, W-only residual (see constants above)
    w2t = _build_w2t(conv_w)
    w16 = 16.0 * w2t[:R]                               # [736, 768]
    W16 = w16.astype(NP_F8)
    Wr16neg = (w16 - W16.astype(np.float32)).astype(NP_F8)  # = -(W16 - 16w2t)
    passes = []
    for Wp in (W16, Wr16neg):
        sw = np.zeros((SCHUNKS * 128, 768), NP_F8)
        sw[:R] = Wp
        passes.append(sw)
    wconv8 = np.zeros((128, NT * 2 * N_DR * 256), NP_F8)
    for t in range(NT):
        for pi in range(2 * N_DR):
            m = MSTART[t] + (pi % N_DR)
            sw = passes[pi // N_DR]
            idx = t * 2 * N_DR + pi
            blk = sw[256 * m : 256 * (m + 1), 128 * t : 128 * (t + 1)]
            wconv8[:, idx * 256 : (idx + 1) * 256] = np.ascontiguousarray(
                blk.reshape(2, 128, 128).transpose(1, 0, 2).reshape(128, 256)
            )

    b1p = att_b1 + C_SELU * (att_w1 @ np.ones(K, np.float32))
    b2p = att_b2 + C_SELU * (att_w2 @ np.ones(U, np.float32))

    wmat = np.zeros((128, 388), np.float32)
    wbias = np.zeros((128, 6), np.float32)
    for q in range(QS):
        sl = slice(32 * q, 32 * q + 32)
        wmat[sl, 0:128][:, sl] = att_w1.T          # w1bd
        wmat[sl, 128:256][:, sl] = att_w2.T        # w2bd
        wmat[sl, 256 + q] = att_w3[0]              # w3bd
        wmat[q, 260 + 32 * q : 260 + 32 * q + 32] = 1.0  # bc4
        wbias[sl, 0] = conv_b + LN_LA
        wbias[sl, 1] = LAM * conv_b
        wbias[sl, 2] = b1p + LN_LA
        wbias[sl, 3] = LAM * b1p
        wbias[sl, 4] = b2p + LN_LA
        wbias[sl, 5] = LAM * b2p
    wmat16 = wmat.astype(NP_BF16)

    # ---------------- per-core inputs + bag bookkeeping
    xf = x.reshape(n_total, R)
    seg_pad = np.concatenate([seg, np.full(n_pad, N_BAGS, seg.dtype)])
    in_maps = []
    bagA = np.zeros((N_CORES, T, QS), np.int64)
    bagB = np.full((N_CORES, T, QS), -1, np.int64)
    npad_sub = np.zeros((N_CORES, T, QS), np.int64)
    for c in range(N_CORES):
        s0 = c * NPAD
        xt = np.zeros((R, NPAD), np.float32)
        real = min(NPAD, max(0, n_total - s0))
        if real > 0:
            xt[:, :real] = xf[s0 : s0 + real].T
        sxr = np.zeros((SCHUNKS * 128, NPAD), NP_F8)
        sxr[:R] = xt.astype(NP_F8)
        sx = np.ascontiguousarray(
            sxr.reshape(SCHUNKS, 128, NPAD).transpose(1, 0, 2).reshape(
                128, SCHUNKS * NPAD
            )
        )
        ids = seg_pad[s0 : s0 + NPAD].reshape(T, QS, FD)
        first = ids[:, :, 0]
        # real-instance last bag per subtile (pad slots hold N_BAGS)
        real_mask = ids < N_BAGS
        last_real = np.where(real_mask, ids, -1).max(axis=2)
        if ((last_real - first) > 1).any() and (last_real >= 0).all():
            raise ValueError("subtile spans >2 bags; unsupported input shape")
        bagA[c] = np.where(first < N_BAGS, first, -1)
        hasB = (last_real > first) & (first < N_BAGS)
        bagB[c] = np.where(hasB, last_real, -1)
        npad_sub[c] = (~real_mask).sum(axis=2)
        maskA = (ids == first[:, :, None]).astype(np.float32)
        maskA *= real_mask  # pad slots excluded even if first is pad
        # layout [QS, T*FD]
        maskp = np.ascontiguousarray(
            maskA.transpose(1, 0, 2).reshape(QS, T * FD)
        ).astype(NP_BF16)
        in_maps.append(
            {
                "sx": sx,
                "wconv": wconv8,
                "wmat": wmat16,
                "wbias": wbias,
                "maskp": maskp,
            }
        )

    nc = _get_program(NPAD)
    trace_mode = int(os.environ.get("DEEPRC_TRACE", "0"))
    kwargs = {}
    if trace_mode == 1:
        kwargs = dict(trace=True, trace_cores=[0])
    elif trace_mode >= 2:
        kwargs = dict(trace=True, trace_cores=list(range(N_CORES)), stitch_traces=True)
    res = run_bass_kernel_spmd(
        nc,
        in_maps,
        core_ids=list(range(N_CORES)),
        **kwargs,
    )
    LAST_RESULTS = res

    # ---------------- pad-instance constants (host, float64-exact path)
    # a zero input row gives er4 = 0 (conv of zeros, max of zeros), so the
    # device computes for each pad slot: e_pad = selu(conv_b)+LA etc.  The
    # unmasked totals z/pooled include those; subtract exactly here.
    if n_pad > 0:
        e_pad = np.where(
            conv_b > 0, LAM * conv_b, LA * np.exp(conv_b) - LA
        ) + LA  # selu(conv_b) + LA, shape [K]
        hh = np.where(
            att_w1 @ (e_pad + C_SELU) + att_b1 > 0,
            LAM * (att_w1 @ (e_pad + C_SELU) + att_b1),
            LA * np.exp(att_w1 @ (e_pad + C_SELU) + att_b1) - LA,
        )
        hh2 = np.where(
            att_w2 @ hh + att_b2 > 0,
            LAM * (att_w2 @ hh + att_b2),
            LA * np.exp(att_w2 @ hh + att_b2) - LA,
        )
        att_pad = float(att_w3[0] @ hh2)
        pz_pad = float(np.exp(att_pad))
        ppool_pad = pz_pad * e_pad  # [K]
    else:
        pz_pad = 0.0
        ppool_pad = np.zeros(K)

    # ---------------- exact host combine (float64)
    Z = np.zeros(N_BAGS, np.float64)
    P = np.zeros((N_BAGS, K), np.float64)
    for c in range(N_CORES):
        r = res.results[c]
        outs = r["outs"].astype(np.float64)          # [128, 4T]
        pooled = outs[:, 0:T].reshape(QS, K, T)
        pooleda = outs[:, T : 2 * T].reshape(QS, K, T)
        z = outs[0:QS, 2 * T : 3 * T]                # [4, T]
        za = outs[0:QS, 3 * T : 4 * T]
        for j in range(T):
            for q in range(QS):
                bA = bagA[c, j, q]
                if bA < 0:
                    continue
                bB = bagB[c, j, q]
                if bB < 0:
                    Z[bA] += za[q, j]
                    P[bA] += pooleda[q, :, j]
                else:
                    Z[bA] += za[q, j]
                    P[bA] += pooleda[q, :, j]
                    npd = npad_sub[c, j, q]
                    Z[bB] += z[q, j] - za[q, j] - npd * pz_pad
                    P[bB] += (
                        pooled[q, :, j] - pooleda[q, :, j] - npd * ppool_pad
                    )

    out = np.zeros((N_BAGS, 1), np.float32)
    for b in range(N_BAGS):
        pooled_bag = P[b] / Z[b] + C_SELU
        out[b, 0] = np.float32(
            float(out_w.astype(np.float64)[0] @ pooled_bag) + float(out_b[0])
        )
    return out
